# revision 1
# baseline (speedup 1.0000x reference)
"""Trainium2 Bass kernel for nn_Block (moe_routing): transformer block =
LN1 + rotary/pos + 16-head causal attention + residual, then LN2 +
top-2-of-8-expert MoE FFN + residual.

Sharding over 8 NeuronCores:
  - attention: head-group sharded. Core c handles batch b=c//4 and head
    pairs {2g, 2g+1} with g=c%4, over ALL T queries of its batch, with
    static causal block skipping (query slice qs only visits key tiles
    kt < 4*(qs+1); only diagonal blocks need a mask, and that mask is the
    same [512,512] pattern for every qs).  Head partials are combined with
    a ReduceScatter over the 4-core group of each batch, in a transposed
    block layout ([4 blocks of [D, 512]]) so each core receives the summed
    x2.T for exactly its 512 owned tokens ([b, 512g:512g+512]).
  - experts: core c owns expert c (expert-parallel MoE).

MoE routing is computed OWNER-side during the attention tail: each core
computes top-2 gating for its own 512 tokens and packs, per expert e, a
single fp32 value val_e = flag * (tokid*2048 + cw*2047 + 1) - 1 into its
AllGather payload row (h2 row bf16 + 8 fp32 routing values, padded to a
2304-byte row so expert cores can dma_gather straight out of the AG
output).  Post-AG per-core routing is just: select my expert's column,
one sparse_gather, and an integer decode (tokid = v>>11, cw = (v&2047)).

All device activations are kept transposed ([D(part), tokens(free)]) so
every matmul contracts over the partition axis.
"""

import math
import os
import sys

import numpy as np

sys.path.insert(0, "/opt/trn_rl_repo")

import concourse.bass as bass  # noqa: E402
import concourse.tile as tile  # noqa: E402
from concourse import bacc, mybir  # noqa: E402
from concourse.alu_op_type import AluOpType  # noqa: E402
from concourse.masks import make_identity  # noqa: E402

AF = mybir.ActivationFunctionType
FP32 = mybir.dt.float32
BF16 = mybir.dt.bfloat16
I32 = mybir.dt.int32
I16 = mybir.dt.int16
P = 128
NCORE = 8
EPS = 1e-5


class Cfg:
    def __init__(self, T=2048, D=1024, H=16, F=4096, CAP=1280, MOE_CHUNK=640):
        self.B = 2
        self.T = T
        self.D = D
        self.H = H
        self.HD = D // H
        self.F = F
        self.E = 8
        self.CAP = CAP
        self.N = self.B * T            # total tokens
        self.TL = self.N // NCORE      # tokens per core
        self.DC = D // P               # D chunks
        self.KT = T // P               # key tiles
        self.TLT = self.TL // P        # local token tiles
        self.PAIRS = H // 2
        self.FT = F // P               # F tiles
        self.CI = CAP // P             # capacity tiles
        self.MOE_CHUNK = MOE_CHUNK     # slots per MoE token chunk
        self.MCN = CAP // MOE_CHUNK    # number of MoE chunks
        self.MCT = MOE_CHUNK // P      # 128-tiles per MoE chunk
        # AG row: h2 bf16 (D elems) + 8 fp32 routing vals (16 bf16-equiv)
        # padded to a multiple of 128 bf16 elems (256 bytes) for dma_gather
        self.ROWW = D + P              # 1152 bf16 = 2304 B = 9*256
        self.RV0 = (2 * D) // 4        # fp32 col offset of routing vals (512)
        assert self.HD == 64 and H % 2 == 0 and self.E == 8
        assert T % 512 == 0 and D % P == 0 and F % P == 0
        assert CAP % MOE_CHUNK == 0 and MOE_CHUNK % P == 0
        assert self.TL % P == 0 and self.N % 16 == 0
        assert (self.ROWW * 2) % 256 == 0


def _nslices(n, step=512):
    return [(i, min(step, n - i)) for i in range(0, n, step)]


def build_nc(cfg: Cfg):
    """Build the SPMD Bass program (same program on all 8 cores)."""
    c = cfg
    nc = bacc.Bacc("TRN2", target_bir_lowering=False, debug=False,
                   num_devices=NCORE)
    RG = [list(range(NCORE))]

    # ---------------- I/O ----------------
    def din(name, shape, dt=FP32):
        return nc.dram_tensor(name, list(shape), dt, kind="ExternalInput").ap()

    LP = 2                                      # local head pairs per core
    xposT = din("xposT", (c.D, c.T), BF16)      # (x+pos).T, bf16, full batch
    xqT = din("xqT", (c.D, c.TL))               # x.T my block (residual)
    wq = din("wq", (LP, P, c.DC, P), BF16)      # my 2 pairs, pre-scaled
    wk = din("wk", (LP, P, c.DC, P), BF16)
    wv = din("wv", (LP, P, c.DC, P), BF16)
    wpT = din("wpT", (c.DC, P, LP, P), BF16)    # w_proj.T rows of my pairs
    bproj = din("bproj", (c.D,))
    ln1g = din("ln1g", (c.D,))
    ln1b = din("ln1b", (c.D,))
    ln2g = din("ln2g", (c.D,))
    ln2b = din("ln2b", (c.D,))
    gwT = din("gwT", (c.D, c.E))
    w1 = din("w1", (c.FT, P, c.DC, P), BF16)
    b1 = din("b1", (c.F,))
    w2 = din("w2", (c.F, c.D), BF16)
    b2 = din("b2", (c.D,))
    maskD = din("maskD", (512, 512), BF16)    # diag-block mask {0,1}
    mbat = din("mbat", (1, 2))                # [is_batch0, is_batch1]
    onehot = din("onehot", (1, c.E))
    tok2048 = din("tok2048", (c.TL,))         # (global tokid)*2048 + 1
    out = nc.dram_tensor("out", [c.TL, c.D], FP32, kind="ExternalOutput").ap()

    # ---------------- internal DRAM ----------------
    NPAD = c.N + 2 * P                      # pad rows for sentinel slots
    HD2 = c.D // 2
    rs_inA = nc.dram_tensor("rs_inA", [NCORE * HD2, 512], BF16).ap()
    rs_inB = nc.dram_tensor("rs_inB", [NCORE * HD2, 512], BF16).ap()
    rs_outA = nc.dram_tensor("rs_outA", [HD2, 512], BF16).ap()
    rs_outB = nc.dram_tensor("rs_outB", [HD2, 512], BF16).ap()
    ag_in = nc.dram_tensor("ag_in", [c.TL, c.ROWW], BF16).ap()
    h2_full = nc.dram_tensor("h2_full", [NPAD, c.ROWW], BF16,
                             addr_space="Shared").ap()
    cwlin = nc.dram_tensor("cwlin", [c.CAP], FP32).ap()
    # MoE combine in 2 column passes of 512: RS#L hides under the R-pass
    # compute; RS#R is exposed. (A 3-pass split was measured SLOWER: the
    # RSs serialize on the ring and per-collective fixed overhead is
    # ~40-80us nearly independent of payload size.)
    MOE_COLS = ((0, 512), (512, 512))
    moe_acc = [nc.dram_tensor(f"moe_acc{i}", [NPAD, w], BF16).ap()
               for i, (_, w) in enumerate(MOE_COLS)]
    moe_s = [nc.dram_tensor(f"moe_s{i}", [c.TL, w], BF16).ap()
             for i, (_, w) in enumerate(MOE_COLS)]

    NT16 = c.N // 16
    CAP16 = c.CAP // 16
    SENT = float(c.N * 2048)   # sentinel routing val: tokid=N, cw=0

    with tile.TileContext(nc) as tc:
        with tc.tile_pool(name="persist", bufs=1) as pp:
            # ---------------- constants ----------------
            ident = pp.tile([P, P], FP32)
            make_identity(nc, ident[:])
            ones = pp.tile([P, 1], FP32)
            nc.vector.memset(ones[:], 1.0)
            ones_bf = pp.tile([P, 1], BF16)
            nc.vector.memset(ones_bf[:], 1.0)

            def load_pcol(ap_dram, n):
                # [n*128] dram -> [128, n] sbuf (per-partition scalars)
                t = pp.tile([P, n], FP32, name=ap_dram.tensor.name + "_sb")
                nc.sync.dma_start(out=t[:], in_=ap_dram.rearrange(
                    "(a p) -> p a", p=P))
                return t

            bproj_sb = load_pcol(bproj, c.DC)
            ln1g_sb = load_pcol(ln1g, c.DC)
            ln1b_sb = load_pcol(ln1b, c.DC)
            ln2g_sb = load_pcol(ln2g, c.DC)
            ln2b_sb = load_pcol(ln2b, c.DC)
            b1_sb = load_pcol(b1, c.FT)

            gwT_sb = pp.tile([P, c.DC, c.E], FP32)
            nc.sync.dma_start(out=gwT_sb[:], in_=gwT.rearrange(
                "(a p) e -> p a e", p=P))
            onehot_sb = pp.tile([P, c.E], FP32)
            nc.sync.dma_start(out=onehot_sb[:], in_=bass.AP(
                tensor=onehot.tensor, offset=onehot.offset,
                ap=[[0, P]] + list(onehot.ap[1:])))
            tok_sb = pp.tile([P, c.TLT], FP32)   # tokid*2048 + 1
            nc.sync.dma_start(out=tok_sb[:], in_=tok2048.rearrange(
                "(tt p) -> p tt", p=P))
            mbat_sb = pp.tile([P, 2], FP32)
            nc.sync.dma_start(out=mbat_sb[:], in_=bass.AP(
                tensor=mbat.tensor, offset=mbat.offset,
                ap=[[0, P]] + list(mbat.ap[1:])))

            x2r_all = pp.tile([P, c.TLT, c.D], FP32)  # x2 rows (for residual)

            # long-lived routing results
            cw_all = pp.tile([P, c.CI], FP32)        # combine weight per slot
            idx_w = pp.tile([P, CAP16], I16)         # wrapped idx, x8

            # =========================================================
            # PHASE A: attention
            # =========================================================
            def ln_core(ctx_pool, ps_pool, xp, ntok, g_sb, b_sb, dt=FP32):
                """LN over partition axis of xp [128, DC, ntok] in place
                (center -> var -> scale), in dtype dt."""
                one_t = ones if dt == FP32 else ones_bf
                stat = ctx_pool.tile([1, ntok], FP32, tag="ln_stat", bufs=1)
                for ns, nn in _nslices(ntok):
                    ps_sum = ps_pool.tile([1, 512], FP32, tag="ln_ps1")
                    for dc in range(c.DC):
                        nc.tensor.matmul(ps_sum[:, :nn], one_t[:],
                                         xp[:, dc, ns:ns + nn],
                                         start=(dc == 0), stop=(dc == c.DC - 1))
                    nc.vector.tensor_copy(out=stat[:, ns:ns + nn],
                                          in_=ps_sum[:, :nn])
                nc.vector.tensor_scalar(out=stat[:], in0=stat[:],
                                        scalar1=1.0 / c.D, scalar2=None,
                                        op0=AluOpType.mult)
                statc = stat
                if dt != FP32:
                    statc = ctx_pool.tile([1, ntok], dt, tag="ln_statc",
                                          bufs=1)
                    nc.vector.tensor_copy(out=statc[:], in_=stat[:])
                # center xp in place (slice-wise broadcast of mu)
                for ns, nn in _nslices(ntok):
                    bc = ctx_pool.tile([P, 512], dt, tag="ln_bc")
                    nc.gpsimd.partition_broadcast(out_ap=bc[:, :nn],
                                                  in_ap=statc[:, ns:ns + nn])
                    for dc in range(c.DC):
                        nc.vector.tensor_tensor(out=xp[:, dc, ns:ns + nn],
                                                in0=xp[:, dc, ns:ns + nn],
                                                in1=bc[:, :nn],
                                                op=AluOpType.subtract)
                # variance of centered values
                stat2 = ctx_pool.tile([1, ntok], FP32, tag="ln_stat2", bufs=1)
                for ns, nn in _nslices(ntok):
                    ps_sq = ps_pool.tile([1, 512], FP32, tag="ln_ps2")
                    for dc in range(c.DC):
                        sq = ctx_pool.tile([P, 512], dt, tag="ln_sq")
                        nc.scalar.activation(out=sq[:, :nn],
                                             in_=xp[:, dc, ns:ns + nn],
                                             func=AF.Square)
                        nc.tensor.matmul(ps_sq[:, :nn], one_t[:], sq[:, :nn],
                                         start=(dc == 0), stop=(dc == c.DC - 1))
                    nc.vector.tensor_copy(out=stat2[:, ns:ns + nn],
                                          in_=ps_sq[:, :nn])
                nc.vector.tensor_scalar(out=stat2[:], in0=stat2[:],
                                        scalar1=1.0 / c.D, scalar2=EPS,
                                        op0=AluOpType.mult, op1=AluOpType.add)
                nc.scalar.activation(out=stat2[:], in_=stat2[:], func=AF.Sqrt)
                nc.vector.reciprocal(out=stat2[:], in_=stat2[:])
                stat2c = stat2
                if dt != FP32:
                    stat2c = ctx_pool.tile([1, ntok], dt, tag="ln_statc",
                                           bufs=1)
                    nc.vector.tensor_copy(out=stat2c[:], in_=stat2[:])
                for ns, nn in _nslices(ntok):
                    bc = ctx_pool.tile([P, 512], dt, tag="ln_bc")
                    nc.gpsimd.partition_broadcast(out_ap=bc[:, :nn],
                                                  in_ap=stat2c[:, ns:ns + nn])
                    for dc in range(c.DC):
                        t = ctx_pool.tile([P, 512], dt, tag="ln_t")
                        nc.vector.tensor_tensor(out=t[:, :nn],
                                                in0=xp[:, dc, ns:ns + nn],
                                                in1=bc[:, :nn],
                                                op=AluOpType.mult)
                        nc.vector.tensor_scalar(out=xp[:, dc, ns:ns + nn],
                                                in0=t[:, :nn],
                                                scalar1=g_sb[:, dc:dc + 1],
                                                scalar2=b_sb[:, dc:dc + 1],
                                                op0=AluOpType.mult,
                                                op1=AluOpType.add)

            def layernorm_T(ctx_pool, ps_pool, x_in, ntok, g_sb, b_sb,
                            out_tile, dt=FP32):
                """In: dram [D, ntok] (pos pre-added, bf16). Out: out_tile
                [128,DC,ntok] layernormed, transposed layout."""
                xp = out_tile
                for dc in range(c.DC):
                    sl = slice(dc * P, (dc + 1) * P)
                    nc.sync.dma_start(out=xp[:, dc, :], in_=x_in[sl, :])
                ln_core(ctx_pool, ps_pool, xp, ntok, g_sb, b_sb, dt=dt)

            with tc.tile_pool(name="attn", bufs=1) as ap_, \
                 tc.tile_pool(name="attn2", bufs=2) as ap2, \
                 tc.tile_pool(name="attn3", bufs=3) as ap3:

                hT = ap_.tile([P, c.DC, c.T], BF16)       # full-batch h (ln1)
                with tc.tile_pool(name="ln_ps", bufs=2, space="PSUM") as lnps:
                    layernorm_T(ap2, lnps, xposT, c.T,
                                ln1g_sb, ln1b_sb, hT, dt=BF16)
                # diag-block causal masks (4 x [128, 512]), loaded once
                mk_sb = ap_.tile([P, 4, 512], BF16, tag="mk")
                nc.sync.dma_start(out=mk_sb[:], in_=maskD.rearrange(
                    "(kl p) q -> p kl q", p=P))

                # zero moe_full + h2_full pad rows via broadcast DMA (early,
                # off the routing critical path)
                zt = ap_.tile([P, c.ROWW], BF16, tag="zt")
                nc.vector.memset(zt[:], 0.0)
                nblk = NPAD // P
                for mtens, (_, w) in zip(moe_acc, MOE_COLS):
                    nc.sync.dma_start(
                        out=mtens.rearrange("(a p) d -> p a d", p=P),
                        in_=bass.AP(tensor=zt[:].tensor, offset=zt[:].offset,
                                    ap=[list(zt[:].ap[0]), [0, nblk],
                                        [1, w]]))
                nc.sync.dma_start(
                    out=h2_full[c.N:NPAD, :].rearrange(
                        "(a p) d -> p a d", p=P),
                    in_=bass.AP(tensor=zt[:].tensor, offset=zt[:].offset,
                                ap=[list(zt[:].ap[0]), [0, 2],
                                    [1, c.ROWW]]))

                catT = ap_.tile([P, 2, c.T], BF16)

                pair_ps = tc.tile_pool(name="pair_ps", bufs=2, space="PSUM")
                pair_ps1 = tc.tile_pool(name="pair_ps1", bufs=1, space="PSUM")
                aps, aps1 = pair_ps.__enter__(), pair_ps1.__enter__()
                for p_ in range(2):
                    wq_t = ap2.tile([P, c.DC, P], BF16, tag="w_t", bufs=3)
                    wk_t = ap2.tile([P, c.DC, P], BF16, tag="w_t", bufs=3)
                    wv_t = ap2.tile([P, c.DC, P], BF16, tag="w_t", bufs=3)
                    nc.sync.dma_start(out=wq_t[:], in_=wq[p_, :, :, :])
                    nc.sync.dma_start(out=wk_t[:], in_=wk[p_, :, :, :])
                    nc.sync.dma_start(out=wv_t[:], in_=wv[p_, :, :, :])

                    # qT for this pair: [128(2 heads x 64), T] (all queries)
                    qT = ap_.tile([P, c.T], BF16, tag="qT", bufs=2)
                    for ns, nn in _nslices(c.T):
                        ps_q = aps1.tile([P, 512], FP32, tag="ps_q")
                        for dc in range(c.DC):
                            nc.tensor.matmul(ps_q[:, :nn], wq_t[:, dc, :],
                                             hT[:, dc, ns:ns + nn],
                                             start=(dc == 0),
                                             stop=(dc == c.DC - 1))
                        nc.vector.tensor_copy(out=qT[:, ns:ns + nn],
                                              in_=ps_q[:, :nn])
                    # kT: [128, T]
                    kT = ap_.tile([P, c.T], BF16, tag="kT", bufs=2)
                    for ns, nn in _nslices(c.T):
                        ps_k = aps.tile([P, 512], FP32, tag="ps_k")
                        for dc in range(c.DC):
                            nc.tensor.matmul(ps_k[:, :nn], wk_t[:, dc, :],
                                             hT[:, dc, ns:ns + nn],
                                             start=(dc == 0),
                                             stop=(dc == c.DC - 1))
                        nc.vector.tensor_copy(out=kT[:, ns:ns + nn],
                                              in_=ps_k[:, :nn])
                    # v rows + ones col: v_aug [128, KT, 2, 66]
                    v_aug = ap_.tile([P, c.KT, 2, 66], BF16, tag="v_aug", bufs=2)
                    nc.vector.memset(v_aug[:, :, :, 64:65], 1.0)
                    for kt in range(c.KT):
                        ps_v = aps1.tile([P, P], FP32, tag="ps_v")
                        ksl = slice(kt * P, (kt + 1) * P)
                        for dc in range(c.DC):
                            nc.tensor.matmul(ps_v[:], hT[:, dc, ksl],
                                             wv_t[:, dc, :],
                                             start=(dc == 0),
                                             stop=(dc == c.DC - 1))
                        nc.vector.tensor_copy(
                            out=v_aug[:, kt, :, 0:64],
                            in_=ps_v[:].rearrange("p (h e) -> p h e", h=2))

                    # causal scores -> (mask on diag blocks) -> exp -> AV,
                    # per 512-query slice qs: only key tiles kt < 4*(qs+1)
                    for qs in range(4):
                        qsl = slice(qs * 512, (qs + 1) * 512)
                        nkt = 4 * (qs + 1)
                        ps_av0 = aps1.tile([65, 512], FP32, tag="ps_av0")
                        ps_av1 = aps1.tile([65, 512], FP32, tag="ps_av1")
                        ps_avs = (ps_av0, ps_av1)
                        for kt in range(nkt):
                            diag = kt >= 4 * qs
                            for h2 in range(2):
                                hsl = slice(h2 * 64, (h2 + 1) * 64)
                                ps_s = aps.tile([P, 512], FP32, tag="ps_s")
                                nc.tensor.matmul(
                                    ps_s[:], kT[hsl, kt * P:(kt + 1) * P],
                                    qT[hsl, qsl], start=True, stop=True)
                                et = ap3.tile([P, 512], BF16, tag="et", bufs=4)
                                nc.scalar.activation(out=et[:], in_=ps_s[:],
                                                     func=AF.Exp)
                                if diag:
                                    nc.vector.tensor_tensor(
                                        out=et[:], in0=et[:],
                                        in1=mk_sb[:, kt - 4 * qs, :],
                                        op=AluOpType.mult)
                                nc.tensor.matmul(ps_avs[h2][:],
                                                 v_aug[:, kt, h2, 0:65], et[:],
                                                 start=(kt == 0),
                                                 stop=(kt == nkt - 1))
                        # normalize; head 2p -> catT rows 0:64 direct,
                        # head 2p+1 -> rows 64:128 via DMA partition shift
                        for h2 in range(2):
                            rec = ap2.tile([1, 512], FP32, tag="rec", bufs=1)
                            nc.vector.reciprocal(out=rec[:],
                                                 in_=ps_avs[h2][64:65, :])
                            rec_bc = ap2.tile([64, 512], FP32, tag="rec_bc",
                                              bufs=1)
                            nc.gpsimd.partition_broadcast(out_ap=rec_bc[:],
                                                          in_ap=rec[:])
                            if h2 == 0:
                                nc.vector.tensor_tensor(
                                    out=catT[0:64, p_, qsl],
                                    in0=ps_avs[0][0:64, :],
                                    in1=rec_bc[:], op=AluOpType.mult)
                            else:
                                shf = ap2.tile([64, 512], BF16, tag="shf",
                                               bufs=1)
                                nc.vector.tensor_tensor(
                                    out=shf[:], in0=ps_avs[1][0:64, :],
                                    in1=rec_bc[:], op=AluOpType.mult)
                                nc.sync.dma_start(out=catT[64:128, p_, qsl],
                                                  in_=shf[:])

                pair_ps1.__exit__(None, None, None)
                pair_ps.__exit__(None, None, None)

                # partial proj over my 2 pairs for ALL T -> rs_in blocks.
                # Global block layout: block (4b + i) = partial x2.T columns
                # of batch b, quarter i. Each core writes quarter i to both
                # batch slots, scaled by its per-core {0,1} batch mask, so
                # the global 8-core ReduceScatter hands core c exactly the
                # summed x2.T of its own tokens (chunk c = block c).
                proj_ps = tc.tile_pool(name="proj_ps", bufs=2, space="PSUM")
                aps = proj_ps.__enter__()
                # split the combine over D halves: RS#A (dco 0-3) issues
                # while proj still computes dco 4-7, and RS#B overlaps the
                # x2/LN2 assembly of the first half
                for dco in range(c.DC):
                    rs_t = rs_inA if dco < 4 else rs_inB
                    dloc = dco % 4
                    wp_t = ap2.tile([P, 2, P], BF16, tag="wp_t", bufs=1)
                    nc.sync.dma_start(out=wp_t[:], in_=wpT[dco, :, :, :])
                    for ns, nn in _nslices(c.T):
                        ps_p = aps.tile([P, 512], FP32, tag="ps_p")
                        for jc in range(2):
                            nc.tensor.matmul(ps_p[:, :nn], wp_t[:, jc, :],
                                             catT[:, jc, ns:ns + nn],
                                             start=(jc == 0),
                                             stop=(jc == 1))
                        for bi in range(2):
                            t = ap2.tile([P, 512], BF16, tag="x2t", bufs=4)
                            # scalar engine: vector is saturated here
                            nc.scalar.activation(
                                out=t[:, :nn], in_=ps_p[:, :nn],
                                func=AF.Copy,
                                scale=mbat_sb[:, bi:bi + 1])
                            blk = 4 * bi + ns // 512
                            nc.sync.dma_start(
                                out=rs_t[blk * HD2 + dloc * P:
                                         blk * HD2 + (dloc + 1) * P, :],
                                in_=t[:, :nn])
                    if dco == 3:
                        nc.gpsimd.collective_compute(
                            "ReduceScatter", AluOpType.add, replica_groups=RG,
                            ins=[rs_inA[:]], outs=[rs_outA[:]])
                proj_ps.__exit__(None, None, None)
                nc.gpsimd.collective_compute(
                    "ReduceScatter", AluOpType.add, replica_groups=RG,
                    ins=[rs_inB[:]], outs=[rs_outB[:]])

                tail_ps = tc.tile_pool(name="tail_ps", bufs=2, space="PSUM")
                aps = tail_ps.__enter__()

                # x2 = attnRS + xq + bproj, into h2T (transposed layout);
                # save x2.T to DRAM for the final residual
                h2T = ap_.tile([P, c.DC, c.TL], FP32)
                for dco in range(c.DC):
                    rs_o = rs_outA if dco < 4 else rs_outB
                    dloc = dco % 4
                    xq_res = ap2.tile([P, c.TL], FP32, tag="xq_res")
                    nc.sync.dma_start(out=xq_res[:],
                                      in_=xqT[dco * P:(dco + 1) * P, :])
                    rso = ap2.tile([P, c.TL], BF16, tag="rso")
                    nc.sync.dma_start(out=rso[:],
                                      in_=rs_o[dloc * P:(dloc + 1) * P, :])
                    t = ap2.tile([P, c.TL], FP32, tag="x2t2", bufs=2)
                    nc.vector.tensor_scalar(
                        out=t[:], in0=rso[:],
                        scalar1=bproj_sb[:, dco:dco + 1], scalar2=None,
                        op0=AluOpType.add)
                    nc.vector.tensor_tensor(
                        out=h2T[:, dco, :], in0=t[:],
                        in1=xq_res[:], op=AluOpType.add)
                    # x2 rows for the final residual: PE transpose now,
                    # before LN2 overwrites h2T in place (avoids a DRAM
                    # round-trip that clogged the DMA queue during routing)
                    for tt in range(c.TLT):
                        ps_t = aps.tile([P, P], FP32, tag="ps_tr")
                        nc.tensor.transpose(
                            out=ps_t[:],
                            in_=h2T[:, dco, tt * P:(tt + 1) * P],
                            identity=ident[:])
                        nc.vector.tensor_copy(
                            out=x2r_all[:, tt, dco * P:(dco + 1) * P],
                            in_=ps_t[:])

                # LN2 in place on h2T (x2 -> h2)
                ln_core(ap2, aps, h2T, c.TL, ln2g_sb, ln2b_sb)

                # gate logits for my block: [128, TLT, E]
                lg_loc = ap_.tile([P, c.TLT, c.E], FP32)
                for tt in range(c.TLT):
                    ps_l = aps.tile([P, c.E], FP32, tag="ps_l")
                    tsl = slice(tt * P, (tt + 1) * P)
                    for dc in range(c.DC):
                        nc.tensor.matmul(ps_l[:], h2T[:, dc, tsl],
                                         gwT_sb[:, dc, :],
                                         start=(dc == 0), stop=(dc == c.DC - 1))
                    nc.vector.tensor_copy(out=lg_loc[:, tt, :], in_=ps_l[:])

                # ---- owner-side top-2 routing for my 512 tokens ----
                # m1 = max_e, m2 = 2nd max, w1 = sigmoid(m1-m2), w2 = 1-w1
                rp1 = ap_  # reuse attn pool for small tiles
                m1 = rp1.tile([P, c.TLT], FP32, tag="rt_m1")
                nc.vector.tensor_reduce(out=m1[:], in_=lg_loc[:],
                                        axis=mybir.AxisListType.X,
                                        op=AluOpType.max)
                eq1 = rp1.tile([P, c.TLT, c.E], FP32, tag="rt_eq1")
                for tt in range(c.TLT):
                    nc.vector.tensor_scalar(out=eq1[:, tt, :],
                                            in0=lg_loc[:, tt, :],
                                            scalar1=m1[:, tt:tt + 1],
                                            scalar2=None,
                                            op0=AluOpType.is_equal)
                tmp = rp1.tile([P, c.TLT, c.E], FP32, tag="rt_tmp")
                nc.vector.tensor_scalar(out=tmp[:], in0=eq1[:],
                                        scalar1=-1e30, scalar2=None,
                                        op0=AluOpType.mult)
                nc.vector.tensor_tensor(out=tmp[:], in0=lg_loc[:], in1=tmp[:],
                                        op=AluOpType.add)
                m2 = rp1.tile([P, c.TLT], FP32, tag="rt_m2")
                nc.vector.tensor_reduce(out=m2[:], in_=tmp[:],
                                        axis=mybir.AxisListType.X,
                                        op=AluOpType.max)
                d12 = rp1.tile([P, c.TLT], FP32, tag="rt_d12")
                nc.vector.tensor_tensor(out=d12[:], in0=m1[:], in1=m2[:],
                                        op=AluOpType.subtract)
                w1q = rp1.tile([P, c.TLT], FP32, tag="rt_w1q")
                nc.scalar.activation(out=w1q[:], in_=d12[:], func=AF.Sigmoid)
                # w1q = w1*2046 ; w2q = (1-w1)*2046 ; dq = w1q - w2q
                # (2046 not 2047 so cwq never carries into the tokid bits
                # even when sigmoid rounds to exactly 1.0 in fp32)
                w2q = rp1.tile([P, c.TLT], FP32, tag="rt_w2q")
                nc.vector.tensor_scalar(out=w2q[:], in0=w1q[:],
                                        scalar1=-2046.0, scalar2=2046.0,
                                        op0=AluOpType.mult,
                                        op1=AluOpType.add)
                nc.vector.tensor_scalar(out=w1q[:], in0=w1q[:],
                                        scalar1=2046.0, scalar2=None,
                                        op0=AluOpType.mult)
                dq = rp1.tile([P, c.TLT], FP32, tag="rt_dq")
                nc.vector.tensor_tensor(out=dq[:], in0=w1q[:], in1=w2q[:],
                                        op=AluOpType.subtract)
                # vals[:, tt, e] = flag_e * (tok2048 + 1 + cwq_e) - 1
                vals = rp1.tile([P, c.TLT, c.E], FP32, tag="rt_vals")
                flg = rp1.tile([P, c.TLT, c.E], FP32, tag="rt_flg")
                for tt in range(c.TLT):
                    # cwq = eq1*dq + w2q  (into vals)
                    nc.vector.tensor_scalar(out=vals[:, tt, :],
                                            in0=eq1[:, tt, :],
                                            scalar1=dq[:, tt:tt + 1],
                                            scalar2=w2q[:, tt:tt + 1],
                                            op0=AluOpType.mult,
                                            op1=AluOpType.add)
                    # += tok2048 + 1
                    nc.vector.tensor_scalar(out=vals[:, tt, :],
                                            in0=vals[:, tt, :],
                                            scalar1=tok_sb[:, tt:tt + 1],
                                            scalar2=None,
                                            op0=AluOpType.add)
                    # flag = lg >= m2
                    nc.vector.tensor_scalar(out=flg[:, tt, :],
                                            in0=lg_loc[:, tt, :],
                                            scalar1=m2[:, tt:tt + 1],
                                            scalar2=None,
                                            op0=AluOpType.is_ge)
                nc.vector.tensor_tensor(out=vals[:], in0=vals[:], in1=flg[:],
                                        op=AluOpType.mult)
                nc.vector.tensor_scalar(out=vals[:], in0=vals[:],
                                        scalar1=-1.0, scalar2=None,
                                        op0=AluOpType.add)
                # write routing vals into ag_in fp32 cols [RV0, RV0+8)
                ag32 = ag_in.bitcast(FP32)
                nc.sync.dma_start(
                    out=ag32[:, c.RV0:c.RV0 + c.E].rearrange(
                        "(tt p) e -> p tt e", p=P),
                    in_=vals[:])

                # h2 rows (token-major bf16) for AllGather: PE transpose,
                # staged in SBUF then shipped with a single DMA
                h2r_all = ap_.tile([P, c.TLT, c.D], BF16, tag="h2r_all")
                for tt in range(c.TLT):
                    for dc in range(c.DC):
                        ps_t = aps.tile([P, P], FP32, tag="ps_tr")
                        nc.tensor.transpose(
                            out=ps_t[:],
                            in_=h2T[:, dc, tt * P:(tt + 1) * P],
                            identity=ident[:])
                        nc.vector.tensor_copy(
                            out=h2r_all[:, tt, dc * P:(dc + 1) * P],
                            in_=ps_t[:])
                nc.sync.dma_start(
                    out=ag_in[:, 0:c.D].rearrange("(tt p) d -> p tt d", p=P),
                    in_=h2r_all[:])
                tail_ps.__exit__(None, None, None)

            # =========================================================
            # PHASE B: AllGather h2 rows + routing vals
            # =========================================================
            nc.gpsimd.collective_compute(
                "AllGather", AluOpType.bypass, replica_groups=RG,
                ins=[ag_in[:]], outs=[h2_full[0:c.N, :]])

            # =========================================================
            # PHASE C: post-AG routing: select my expert column, compact,
            # decode. Also x2 row transposes (fill the collective bubble)
            # and the w2 prefetch (overlaps the AllGather).
            # =========================================================
            moe_pp = tc.tile_pool(name="moe_pp", bufs=1)
            mp1 = moe_pp.__enter__()
            w2_sb = mp1.tile([P, c.FT, c.D], BF16)
            nc.sync.dma_start(out=w2_sb[:], in_=w2.rearrange(
                "(o p) d -> p o d", p=P))
            b2_sb = mp1.tile([P, c.D], FP32)
            nc.sync.dma_start(out=b2_sb[:], in_=bass.AP(
                tensor=b2.tensor, offset=b2.offset,
                ap=[[0, P]] + list(b2.ap)))
            with tc.tile_pool(name="route", bufs=1) as rp:
                # my expert's routing vals: select over the 8 fp32 columns
                h2f = h2_full.bitcast(FP32)
                lgsel = rp.tile([16, NT16, c.E], FP32)
                nc.sync.dma_start(out=lgsel[:], in_=h2f[
                    0:c.N, c.RV0:c.RV0 + c.E].rearrange(
                    "(a p) e -> p a e", p=16))
                # val = sum_e val_e * onehot_e  (others contribute 0)
                sgin = rp.tile([16, NT16 + CAP16], FP32)
                nc.vector.memset(sgin[:, NT16:], SENT)
                ohap = onehot_sb[0:16, :]
                ohbc = bass.AP(tensor=ohap.tensor, offset=ohap.offset,
                               ap=[list(ohap.ap[0]), [0, NT16], [1, c.E]])
                nc.vector.tensor_tensor(out=lgsel[:], in0=lgsel[:],
                                        in1=ohbc, op=AluOpType.mult)
                nc.vector.tensor_reduce(out=sgin[:, 0:NT16], in_=lgsel[:],
                                        axis=mybir.AxisListType.X,
                                        op=AluOpType.add)
                # compact: one sparse_gather over the packed values
                selfull = pp.tile([16, NT16 + CAP16], FP32)
                nf1 = rp.tile([1, 1], mybir.dt.uint32)
                nc.gpsimd.sparse_gather(out=selfull[:], in_=sgin[:],
                                        num_found=nf1[:])
                # decode: v = int(val); tok = v >> 11 ; cw = (v & 2047)/2046
                v32 = pp.tile([16, CAP16], I32)
                nc.vector.tensor_copy(out=v32[:], in_=selfull[:, 0:CAP16])
                t32 = rp.tile([16, CAP16], I32)
                nc.vector.tensor_scalar(out=t32[:], in0=v32[:],
                                        scalar1=11, scalar2=None,
                                        op0=AluOpType.logical_shift_right)
                idsel16 = rp.tile([16, CAP16], I16)
                nc.vector.tensor_copy(out=idsel16[:], in_=t32[:])
                # idx: replicate to the 8 gpsimd core groups (SBUF->SBUF,
                # no DRAM round-trip; the 8 DMAs run concurrently).
                # Emitted before the cw decode so the expert gather isn't
                # queued behind the cw DMAs on the sync engine.
                for g in range(8):
                    nc.sync.dma_start(out=idx_w[g * 16:(g + 1) * 16, :],
                                      in_=idsel16[:])
                cw32 = pp.tile([16, CAP16], I32)
                nc.vector.tensor_scalar(out=cw32[:], in0=v32[:],
                                        scalar1=2047, scalar2=None,
                                        op0=AluOpType.bitwise_and)
                cwf = pp.tile([16, CAP16], FP32)
                nc.vector.tensor_copy(out=cwf[:], in_=cw32[:])
                nc.vector.tensor_scalar(out=cwf[:], in0=cwf[:],
                                        scalar1=1.0 / 2046.0, scalar2=None,
                                        op0=AluOpType.mult)
                # cw: wrapped -> slot-major [128, CI] via DRAM
                nc.sync.dma_start(out=bass.AP(
                    tensor=cwlin.tensor, offset=cwlin.offset,
                    ap=[[1, 16], [16, CAP16]]), in_=cwf[:])
                nc.sync.dma_start(out=cw_all[:], in_=bass.AP(
                    tensor=cwlin.tensor, offset=cwlin.offset,
                    ap=[[1, P], [P, c.CI]]))

            # =========================================================
            # PHASE D: expert FFN over CAP slots in chunks (bf16)
            # =========================================================
            with tc.tile_pool(name="moe2", bufs=2) as mp2, \
                 tc.tile_pool(name="moe_ps", bufs=2, space="PSUM") as mps:
                MC16 = c.MOE_CHUNK // 16
                hidTs = []

                def ffn2_cols(mc, pi):
                    # FFN2 for one (chunk, D-column pass) + bias + cw
                    # scale, then one scatter-add
                    csl = slice(mc * MC16, (mc + 1) * MC16)
                    ns0, w = MOE_COLS[pi]
                    orow = mp2.tile([P, c.MCT, w], BF16,
                                    tag=f"orow{pi}", bufs=1)
                    for mt in range(c.MCT):
                        slotcol = mc * c.MCT + mt
                        ps_o = mps.tile([P, 512], FP32, tag="ps_o")
                        for fc in range(c.FT):
                            nc.tensor.matmul(
                                ps_o[:, :w],
                                hidTs[mc][:, fc, mt * P:(mt + 1) * P],
                                w2_sb[:, fc, ns0:ns0 + w],
                                start=(fc == 0), stop=(fc == c.FT - 1))
                        t = mp2.tile([P, 512], FP32, tag="ot")
                        nc.vector.tensor_tensor(
                            out=t[:, :w], in0=ps_o[:, :w],
                            in1=b2_sb[:, ns0:ns0 + w],
                            op=AluOpType.add)
                        nc.vector.tensor_scalar(
                            out=orow[:, mt, :], in0=t[:, :w],
                            scalar1=cw_all[:, slotcol:slotcol + 1],
                            scalar2=None, op0=AluOpType.mult)
                    nc.gpsimd.dma_scatter_add(
                        out_ap=moe_acc[pi][:], in_ap=orow[:],
                        idxs_ap=idx_w[:, csl],
                        num_idxs=c.MOE_CHUNK, num_idxs_reg=c.MOE_CHUNK,
                        elem_size=w)

                # pass 1: per chunk, FFN1 then the LEFT half of FFN2 +
                # scatter. Once both chunks' left scatters are done, the
                # left ReduceScatter runs while BOTH chunks' right-half
                # FFN2 (pass 2) still compute -- fully hiding it.
                for mc in range(c.MCN):
                    csl = slice(mc * MC16, (mc + 1) * MC16)
                    hsel = mp2.tile([P, c.DC, c.MOE_CHUNK], BF16,
                                    tag=f"hsel{mc}", bufs=1)
                    nc.gpsimd.dma_gather(
                        out_ap=hsel[:], in_ap=h2_full[:, 0:c.D],
                        idxs_ap=idx_w[:, csl],
                        num_idxs=c.MOE_CHUNK, num_idxs_reg=c.MOE_CHUNK,
                        elem_size=c.D, elem_step=c.ROWW, transpose=True)
                    # FFN1: hidT[ft] = relu(w1[:,ft].T @ hsel + b1[ft])
                    hidT = mp2.tile([P, c.FT, c.MOE_CHUNK], BF16,
                                    tag=f"hidT{mc}", bufs=1)
                    hidTs.append(hidT)
                    for ft in range(c.FT):
                        w1t = mp2.tile([P, c.DC, P], BF16, tag="w1t", bufs=3)
                        nc.sync.dma_start(out=w1t[:], in_=w1[ft, :, :, :])
                        for ns, nn in _nslices(c.MOE_CHUNK):
                            ps_h = mps.tile([P, 512], FP32, tag="ps_h")
                            for dc in range(c.DC):
                                nc.tensor.matmul(ps_h[:, :nn],
                                                 w1t[:, dc, :],
                                                 hsel[:, dc, ns:ns + nn],
                                                 start=(dc == 0),
                                                 stop=(dc == c.DC - 1))
                            nc.scalar.activation(
                                out=hidT[:, ft, ns:ns + nn], in_=ps_h[:, :nn],
                                func=AF.Relu, bias=b1_sb[:, ft:ft + 1])
                    ffn2_cols(mc, 0)
                nc.gpsimd.collective_compute(
                    "ReduceScatter", AluOpType.add, replica_groups=RG,
                    ins=[moe_acc[0][0:c.N, :]], outs=[moe_s[0][:]])
                # pass 2 (overlaps RS#L)
                for mc in range(c.MCN):
                    ffn2_cols(mc, 1)
            moe_pp.__exit__(None, None, None)

            # =========================================================
            # PHASE E: last column-pass ReduceScatter + final residual
            # =========================================================
            nc.gpsimd.collective_compute(
                "ReduceScatter", AluOpType.add, replica_groups=RG,
                ins=[moe_acc[1][0:c.N, :]], outs=[moe_s[1][:]])

            with tc.tile_pool(name="fin", bufs=2) as fp:
                for tt in range(c.TLT):
                    for pi, (ns0, w) in enumerate(MOE_COLS):
                        ms = fp.tile([P, 512], BF16, tag="ms")
                        nc.sync.dma_start(
                            out=ms[:, :w],
                            in_=moe_s[pi][tt * P:(tt + 1) * P, :])
                        msf = fp.tile([P, 512], FP32, tag="msf")
                        nc.vector.tensor_copy(out=msf[:, :w], in_=ms[:, :w])
                        orow = fp.tile([P, 512], FP32, tag="fout")
                        nc.vector.tensor_tensor(
                            out=orow[:, :w], in0=x2r_all[:, tt, ns0:ns0 + w],
                            in1=msf[:, :w], op=AluOpType.add)
                        nc.sync.dma_start(
                            out=out[tt * P:(tt + 1) * P, ns0:ns0 + w],
                            in_=orow[:, :w])

    nc.compile()
    return nc


# =====================================================================
# Host side
# =====================================================================

def _rot_table(T, D):
    freqs = (np.arange(0, D, 2, dtype=np.float64) / D)
    t = np.arange(T, dtype=np.float64)
    ang = 2.0 * math.pi * t[:, None] * freqs[None, :]
    rot = np.stack([np.sin(ang), np.cos(ang)], axis=-1).reshape(T, D)
    return rot.astype(np.float32)


def _wtile(w, P_, nI, nO):
    # [nI*128, nO*128] -> [nO, 128, nI, 128]: tile (o) is a contiguous
    # [128p, nI, 128m] block (partition-major rows for single-descriptor DMA)
    return np.ascontiguousarray(
        w.reshape(nI, P_, nO, P_).transpose(2, 1, 0, 3))


def make_in_maps(cfg, x, pos_emb, wq, wk, wv, w_proj, b_proj, ln1_g, ln1_b,
                 ln2_g, ln2_b, gate_w, e_w1, e_b1, e_w2, e_b2):
    import ml_dtypes
    bf16 = ml_dtypes.bfloat16
    c = cfg
    f32 = np.float32
    x = np.asarray(x, f32)
    rot = _rot_table(c.T, c.D)
    pemb = np.asarray(pos_emb, f32)[:c.T]
    scale = c.D ** (-0.5)
    wq_cat = (np.asarray(wq, f32).transpose(1, 0, 2).reshape(c.D, c.D)
              * scale).copy()
    wk_cat = np.asarray(wk, f32).transpose(1, 0, 2).reshape(c.D, c.D).copy()
    wv_cat = np.asarray(wv, f32).transpose(1, 0, 2).reshape(c.D, c.D).copy()
    wpT = np.asarray(w_proj, f32).T.copy()
    gwT = np.asarray(gate_w, f32).T.copy()
    posT = np.ascontiguousarray((rot + pemb).T.astype(f32))
    wq_r = _wtile(wq_cat, P, c.DC, c.PAIRS).astype(bf16)
    wk_r = _wtile(wk_cat, P, c.DC, c.PAIRS).astype(bf16)
    wv_r = _wtile(wv_cat, P, c.DC, c.PAIRS).astype(bf16)
    wp_r = _wtile(wpT, P, c.DC, c.DC).astype(bf16)

    # diag-block causal mask: key (kl*128+r) visible to query q iff
    # kl*128+r <= q  (same pattern for every 512-query slice)
    kk = np.arange(512)
    maskD = np.where(kk[:, None] <= kk[None, :], 1.0, 0.0).astype(bf16)

    in_maps = []
    for core in range(NCORE):
        b, g = core // (NCORE // c.B), core % (NCORE // c.B)
        t0 = g * c.TL
        tok = np.arange(t0, t0 + c.TL)
        onehot = np.zeros((1, c.E), f32)
        onehot[0, core % c.E] = 1.0
        xposT = (x[b].T + posT).astype(bf16)
        gtok = (b * c.T + tok).astype(np.float64)
        m = {
            "xposT": np.ascontiguousarray(xposT),
            "xqT": x[b, t0:t0 + c.TL].T.copy(),
            "wq": np.ascontiguousarray(wq_r[2 * g:2 * g + 2]),
            "wk": np.ascontiguousarray(wk_r[2 * g:2 * g + 2]),
            "wv": np.ascontiguousarray(wv_r[2 * g:2 * g + 2]),
            "wpT": np.ascontiguousarray(wp_r[:, :, 2 * g:2 * g + 2, :]),
            "bproj": np.asarray(b_proj, f32),
            "ln1g": np.asarray(ln1_g, f32), "ln1b": np.asarray(ln1_b, f32),
            "ln2g": np.asarray(ln2_g, f32), "ln2b": np.asarray(ln2_b, f32),
            "gwT": gwT,
            "w1": _wtile(np.asarray(e_w1, f32)[core % c.E], P, c.DC,
                         c.FT).astype(bf16),
            "b1": np.asarray(e_b1, f32)[core % c.E].copy(),
            "w2": np.ascontiguousarray(
                np.asarray(e_w2, f32)[core % c.E]).astype(bf16),
            "b2": np.asarray(e_b2, f32)[core % c.E].copy(),
            "maskD": maskD,
            "mbat": np.array([[1.0 - b, float(b)]], f32),
            "onehot": onehot,
            "tok2048": (gtok * 2048.0 + 1.0).astype(f32),
        }
        in_maps.append(m)
    return in_maps


_CACHE = {}
LAST_RESULTS = None


def _ensure_ntff_hook():
    """Inject antenv.axon_hooks (missing from this image) and install the
    ctypes NTFF profile hook against libaxon_pjrt.so so that
    run_bass_kernel_spmd(trace=True) can capture device profiles."""
    import contextlib
    import ctypes
    import types

    try:
        from antenv.axon_hooks import get_axon_ntff_profile_hook  # noqa: F401
        return True
    except ImportError:
        pass
    so_path = "/opt/axon/libaxon_pjrt.so"
    if not os.path.exists(so_path):
        return False
    lib = ctypes.CDLL(so_path)
    if not hasattr(lib, "axon_start_nrt_profile"):
        return False
    lib.axon_start_nrt_profile.argtypes = [ctypes.POINTER(ctypes.c_int64),
                                           ctypes.c_size_t]
    lib.axon_start_nrt_profile.restype = ctypes.c_int64
    lib.axon_stop_nrt_profile.argtypes = [ctypes.c_char_p]
    lib.axon_stop_nrt_profile.restype = ctypes.c_int64

    @contextlib.contextmanager
    def _hook(output_dir, device_ids):
        import jax
        jax.devices()
        if device_ids:
            ids = (ctypes.c_int64 * len(device_ids))(*device_ids)
            rc = lib.axon_start_nrt_profile(ids, len(device_ids))
        else:
            rc = lib.axon_start_nrt_profile(None, 0)
        if rc != 0:
            raise RuntimeError(f"axon_start_nrt_profile rc={rc}")
        try:
            yield
        finally:
            n = lib.axon_stop_nrt_profile(str(output_dir).encode())
            print(f"ntff profile: {n} file(s) -> {output_dir}",
                  file=sys.stderr)

    mod = types.ModuleType("antenv.axon_hooks")
    state = {"h": _hook}
    mod.set_axon_ntff_profile_hook = lambda h: state.__setitem__("h", h)
    mod.get_axon_ntff_profile_hook = lambda: state["h"]
    sys.modules["antenv.axon_hooks"] = mod
    import antenv
    antenv.axon_hooks = mod
    # avoid remote artifact upload in this container
    from concourse import bass_utils as _bu
    _bu.upload_artifacts = lambda tmpdir: tmpdir
    return True


def kernel(**inputs):
    """Full inputs in (as reference.setup_inputs), full output out."""
    cfg = Cfg()
    key = "full"
    if key not in _CACHE:
        _CACHE[key] = build_nc(cfg)
    nc = _CACHE[key]
    in_maps = make_in_maps(cfg, **{k: np.asarray(v) for k, v in inputs.items()})
    trace = bool(os.environ.get("KB_TRACE"))
    if trace:
        trace = _ensure_ntff_hook()
    from concourse.bass_utils import run_bass_kernel_spmd
    global LAST_RESULTS
    res = run_bass_kernel_spmd(nc, in_maps, list(range(NCORE)), trace=trace)
    LAST_RESULTS = res
    outs = [res.results[i]["out"] for i in range(NCORE)]
    c = cfg
    out = np.zeros((c.B, c.T, c.D), np.float32)
    for core in range(NCORE):
        b, j = core // (NCORE // c.B), core % (NCORE // c.B)
        out[b, j * c.TL:(j + 1) * c.TL] = outs[core]
    return out



# revision 17
# speedup vs baseline: 1.2022x; 1.2022x over previous
"""Trainium2 Bass kernel for nn_Block (moe_routing): transformer block =
LN1 + rotary/pos + 16-head causal attention + residual, then LN2 +
top-2-of-8-expert MoE FFN + residual.

Sharding over 8 NeuronCores:
  - attention: head-group sharded. Core c handles batch b=c//4 and head
    pairs {2g, 2g+1} with g=c%4, over ALL T queries of its batch, with
    static causal block skipping.  Head partials are combined with a
    single 8-core AllToAll of catT quarters (every core writes its
    quarter data into BOTH batch chunk slots; the receiving core's
    per-core projection weights are zero-padded for the foreign group's
    chunks, so the full w_proj contraction over 16 128-chunks yields
    exactly its own batch group's head sum).  Each core then does the
    FULL output projection locally for its 512 owned tokens.
  - experts: core c owns expert c (expert-parallel MoE, CAP=1152 slots).

MoE routing is owner-side: each core computes top-2 gating for its own
512 tokens, packs per expert e a single fp32 value
val_e = flag * (tokid*2048 + cw*2046 + 1) - 1, and AllGathers just the
[N, 8] routing values in a tiny collective issued BEFORE the big h2-row
AllGather.  The whole per-expert routing decode (select column, one
sparse_gather, integer decode) then overlaps the big AllGather, as do
the x2-row transposes for the final residual and the w2 prefetch.

All device activations are kept transposed ([D(part), tokens(free)]) so
every matmul contracts over the partition axis.
"""

import math
import os
import sys

import numpy as np

sys.path.insert(0, "/opt/trn_rl_repo")

import concourse.bass as bass  # noqa: E402
import concourse.tile as tile  # noqa: E402
from concourse import bacc, mybir  # noqa: E402
from concourse.alu_op_type import AluOpType  # noqa: E402
from concourse.masks import make_identity  # noqa: E402

AF = mybir.ActivationFunctionType
FP32 = mybir.dt.float32
BF16 = mybir.dt.bfloat16
I32 = mybir.dt.int32
I16 = mybir.dt.int16
P = 128
NCORE = 8
EPS = 1e-5


class Cfg:
    def __init__(self, T=2048, D=1024, H=16, F=4096, CAP=1152, MOE_CHUNK=384):
        self.B = 2
        self.T = T
        self.D = D
        self.H = H
        self.HD = D // H
        self.F = F
        self.E = 8
        self.CAP = CAP
        self.N = self.B * T            # total tokens
        self.TL = self.N // NCORE      # tokens per core
        self.DC = D // P               # D chunks
        self.KT = T // P               # key tiles
        self.TLT = self.TL // P        # local token tiles
        self.FT = F // P               # F tiles
        self.CI = CAP // P             # capacity tiles
        self.MOE_CHUNK = MOE_CHUNK     # slots per MoE token chunk
        self.MCN = CAP // MOE_CHUNK    # number of MoE chunks
        self.MCT = MOE_CHUNK // P      # 128-tiles per MoE chunk
        assert self.HD == 64 and H % 2 == 0 and self.E == 8
        assert T % 512 == 0 and D % P == 0 and F % P == 0
        assert CAP % MOE_CHUNK == 0 and MOE_CHUNK % P == 0
        assert self.TL % P == 0 and self.N % 16 == 0


def _nslices(n, step=512):
    return [(i, min(step, n - i)) for i in range(0, n, step)]


def build_nc(cfg: Cfg):
    """Build the SPMD Bass program (same program on all 8 cores)."""
    c = cfg
    nc = bacc.Bacc("TRN2", target_bir_lowering=False, debug=False,
                   num_devices=NCORE)
    RG = [list(range(NCORE))]

    # ---------------- I/O ----------------
    def din(name, shape, dt=FP32):
        return nc.dram_tensor(name, list(shape), dt, kind="ExternalInput").ap()

    LP = 2                                      # local head pairs per core
    xposT = din("xposT", (c.D, c.T), BF16)      # (x+pos).T, bf16, full batch
    xqT = din("xqT", (c.D, c.TL))               # x.T my block (residual)
    wq = din("wq", (LP, P, c.DC, P), BF16)      # my 2 pairs, pre-scaled
    wk = din("wk", (LP, P, c.DC, P), BF16)
    wv = din("wv", (LP, P, c.DC, P), BF16)
    # full w_proj.T, packed per (do, ci16): ci chunks of the foreign batch
    # group are ZERO so the 16-chunk contraction of a2a_out picks exactly
    # this core's batch-group head sum.
    wpT = din("wpT", (c.DC, P, 2 * c.DC, P), BF16)
    bproj = din("bproj", (c.D,))
    ln1g = din("ln1g", (c.D,))
    ln1b = din("ln1b", (c.D,))
    ln2g = din("ln2g", (c.D,))
    ln2b = din("ln2b", (c.D,))
    gwT = din("gwT", (c.D, c.E))
    w1 = din("w1", (c.FT, P, c.DC, P), BF16)
    b1 = din("b1", (c.F,))
    w2 = din("w2", (c.F, c.D), BF16)
    b2 = din("b2", (c.D,))
    maskD = din("maskD", (512, 512), BF16)    # diag-block mask {0,1}
    onehot = din("onehot", (1, c.E))
    gconst = din("gconst", (1, c.E))          # gate bias: ln2_b @ gate_w.T
    tok2048 = din("tok2048", (c.TL,))         # (global tokid)*2048 + 1
    out = nc.dram_tensor("out", [c.TL, c.D], FP32, kind="ExternalOutput").ap()

    # ---------------- internal DRAM ----------------
    NPAD = c.N + 2 * P                      # pad rows for sentinel slots
    # AllToAll: chunk j (256 rows) at core g = core g's catT columns for
    # batch-local quarter (j%4).  Every core writes both j and j+4.
    a2a_in = nc.dram_tensor("a2a_in", [NCORE * 2 * P, 512], BF16).ap()
    a2a_out = nc.dram_tensor("a2a_out", [NCORE * 2 * P, 512], BF16).ap()
    rv_in = nc.dram_tensor("rv_in", [c.TL, c.E], FP32).ap()
    rv_full = nc.dram_tensor("rv_full", [c.N, c.E], FP32,
                             addr_space="Shared").ap()
    ag_in = nc.dram_tensor("ag_in", [c.TL, c.D], BF16).ap()
    h2_full = nc.dram_tensor("h2_full", [NPAD, c.D], BF16,
                             addr_space="Shared").ap()
    cwlin = nc.dram_tensor("cwlin", [c.CAP], FP32).ap()
    # MoE combine in 2 column passes of 512: RS#L hides under the R-pass
    # compute; RS#R is partly hidden by the left final-residual work.
    MOE_COLS = ((0, 512), (512, 512))
    moe_acc = [nc.dram_tensor(f"moe_acc{i}", [NPAD, w], BF16).ap()
               for i, (_, w) in enumerate(MOE_COLS)]
    moe_s = [nc.dram_tensor(f"moe_s{i}", [c.TL, w], BF16).ap()
             for i, (_, w) in enumerate(MOE_COLS)]

    NT16 = c.N // 16
    CAP16 = c.CAP // 16
    SENT = float(c.N * 2048)   # sentinel routing val: tokid=N, cw=0

    with tile.TileContext(nc) as tc:
        with tc.tile_pool(name="persist", bufs=1) as pp:
            # ---------------- constants ----------------
            ident = pp.tile([P, P], FP32)
            make_identity(nc, ident[:])
            ident_bf = pp.tile([P, P], BF16)
            make_identity(nc, ident_bf[:])
            ones_bf = pp.tile([P, 1], BF16)
            nc.vector.memset(ones_bf[:], 1.0)

            def load_pcol(ap_dram, n):
                # [n*128] dram -> [128, n] sbuf (per-partition scalars)
                t = pp.tile([P, n], FP32, name=ap_dram.tensor.name + "_sb")
                nc.sync.dma_start(out=t[:], in_=ap_dram.rearrange(
                    "(a p) -> p a", p=P))
                return t

            bproj_sb = load_pcol(bproj, c.DC)
            ln1g_sb = load_pcol(ln1g, c.DC)
            ln1b_sb = load_pcol(ln1b, c.DC)
            ln2g_sb = load_pcol(ln2g, c.DC)
            ln2b_sb = load_pcol(ln2b, c.DC)
            b1_sb = load_pcol(b1, c.FT)

            gwT_sb = pp.tile([P, c.DC, c.E], FP32)
            nc.sync.dma_start(out=gwT_sb[:], in_=gwT.rearrange(
                "(a p) e -> p a e", p=P))
            onehot_sb = pp.tile([P, c.E], FP32)
            nc.sync.dma_start(out=onehot_sb[:], in_=bass.AP(
                tensor=onehot.tensor, offset=onehot.offset,
                ap=[[0, P]] + list(onehot.ap[1:])))
            gconst_sb = pp.tile([P, c.E], FP32)
            nc.sync.dma_start(out=gconst_sb[:], in_=bass.AP(
                tensor=gconst.tensor, offset=gconst.offset,
                ap=[[0, P]] + list(gconst.ap[1:])))
            tok_sb = pp.tile([P, c.TLT], FP32)   # tokid*2048 + 1
            nc.sync.dma_start(out=tok_sb[:], in_=tok2048.rearrange(
                "(tt p) -> p tt", p=P))

            x2r_all = pp.tile([P, c.TLT, c.D], FP32)  # x2 rows (for residual)

            # long-lived routing results
            cw_all = pp.tile([P, c.CI], FP32)        # combine weight per slot
            idx_w = pp.tile([P, CAP16], I16)         # wrapped idx, x8

            # =========================================================
            # PHASE A: attention
            # =========================================================
            with tc.tile_pool(name="attn2", bufs=2) as ap2, \
                 tc.tile_pool(name="attn3", bufs=3) as ap3, \
                 tc.tile_pool(name="attn_ps", bufs=2, space="PSUM") as aps:
                # PSUM budget (8 banks): tag pa = 2 x [128,1024] fp32
                # (2 banks each), tags pb/pc = 2 x [128,512] (1 bank each).
                attn_a = tc.tile_pool(name="attn_a", bufs=1)
                ap_ = attn_a.__enter__()

                # ---- LN1 over the full batch, one-pass stats ----
                # xposT loads are emitted FIRST so the stat matmuls can
                # start as soon as tiles land.
                hT = ap_.tile([P, c.DC, c.T], BF16)   # becomes h (in place)
                for dc in range(c.DC):
                    nc.sync.dma_start(
                        out=hT[:, dc, :],
                        in_=xposT[dc * P:(dc + 1) * P, :])

                stat_s = ap_.tile([1, c.T], FP32, tag="ln_ss")
                stat_q = ap_.tile([1, c.T], FP32, tag="ln_sq")
                for ns, nn in _nslices(c.T):
                    ps_sum = aps.tile([1, 512], FP32, tag="pb")
                    ps_sq = aps.tile([1, 512], FP32, tag="pc")
                    for dc in range(c.DC):
                        nc.tensor.matmul(ps_sum[:, :nn], ones_bf[:],
                                         hT[:, dc, ns:ns + nn],
                                         start=(dc == 0), stop=(dc == c.DC - 1))
                        sq = ap2.tile([P, 512], BF16, tag="ln_sqt")
                        nc.vector.tensor_tensor(out=sq[:, :nn],
                                                in0=hT[:, dc, ns:ns + nn],
                                                in1=hT[:, dc, ns:ns + nn],
                                                op=AluOpType.mult)
                        nc.tensor.matmul(ps_sq[:, :nn], ones_bf[:], sq[:, :nn],
                                         start=(dc == 0), stop=(dc == c.DC - 1))
                    nc.vector.tensor_copy(out=stat_s[:, ns:ns + nn],
                                          in_=ps_sum[:, :nn])
                    nc.vector.tensor_copy(out=stat_q[:, ns:ns + nn],
                                          in_=ps_sq[:, :nn])

                def ln_stats_finish(stat_s, stat_q, ntok, sp):
                    """stat_s <- mean (fp32), returns (mu_bf, r_bf) bf16."""
                    nc.vector.tensor_scalar(out=stat_s[:], in0=stat_s[:],
                                            scalar1=1.0 / c.D, scalar2=None,
                                            op0=AluOpType.mult)
                    musq = sp.tile([1, ntok], FP32, tag="ln_musq", bufs=1)
                    nc.vector.tensor_tensor(out=musq[:], in0=stat_s[:],
                                            in1=stat_s[:], op=AluOpType.mult)
                    # var + eps = E[x^2] + eps - mu^2
                    nc.vector.tensor_scalar(out=stat_q[:], in0=stat_q[:],
                                            scalar1=1.0 / c.D, scalar2=EPS,
                                            op0=AluOpType.mult,
                                            op1=AluOpType.add)
                    nc.vector.tensor_tensor(out=stat_q[:], in0=stat_q[:],
                                            in1=musq[:],
                                            op=AluOpType.subtract)
                    nc.scalar.activation(out=stat_q[:], in_=stat_q[:],
                                         func=AF.Sqrt)
                    nc.vector.reciprocal(out=stat_q[:], in_=stat_q[:])
                    mu_bf = sp.tile([1, ntok], BF16, tag="ln_mubf", bufs=1)
                    r_bf = sp.tile([1, ntok], BF16, tag="ln_rbf", bufs=1)
                    nc.vector.tensor_copy(out=mu_bf[:], in_=stat_s[:])
                    nc.vector.tensor_copy(out=r_bf[:], in_=stat_q[:])
                    return mu_bf, r_bf

                mu_bf, r_bf = ln_stats_finish(stat_s, stat_q, c.T, ap_)

                # apply: h = ((x - mu) * r) * g + b, in place on hT (bf16)
                for ns, nn in _nslices(c.T):
                    mubc = ap2.tile([P, 512], BF16, tag="ln_mubc")
                    rbc = ap2.tile([P, 512], BF16, tag="ln_rbc")
                    nc.gpsimd.partition_broadcast(out_ap=mubc[:, :nn],
                                                  in_ap=mu_bf[:, ns:ns + nn])
                    nc.gpsimd.partition_broadcast(out_ap=rbc[:, :nn],
                                                  in_ap=r_bf[:, ns:ns + nn])
                    for dc in range(c.DC):
                        t = ap2.tile([P, 512], BF16, tag="ln_t", bufs=3)
                        nc.vector.tensor_tensor(out=t[:, :nn],
                                                in0=hT[:, dc, ns:ns + nn],
                                                in1=mubc[:, :nn],
                                                op=AluOpType.subtract)
                        nc.vector.tensor_tensor(out=t[:, :nn], in0=t[:, :nn],
                                                in1=rbc[:, :nn],
                                                op=AluOpType.mult)
                        # scalar engine: out = scale*in + bias
                        nc.scalar.activation(out=hT[:, dc, ns:ns + nn],
                                             in_=t[:, :nn], func=AF.Identity,
                                             scale=ln1g_sb[:, dc:dc + 1],
                                             bias=ln1b_sb[:, dc:dc + 1])

                # diag-block causal masks (4 x [128, 512]), loaded once
                mk_sb = ap_.tile([P, 4, 512], BF16, tag="mk")
                nc.sync.dma_start(out=mk_sb[:], in_=maskD.rearrange(
                    "(kl p) q -> p kl q", p=P))

                catT = ap_.tile([P, 2, c.T], BF16)

                for p_ in range(2):
                    if p_ == 1:
                        # zero moe_acc + h2_full pad rows via broadcast DMA
                        # (during the attention main loop; DMA is idle and
                        # pair-0's weight loads have already gone out)
                        zt = ap_.tile([P, c.D], BF16, tag="zt")
                        nc.vector.memset(zt[:], 0.0)
                        nblk = NPAD // P
                        for mtens, (_, w) in zip(moe_acc, MOE_COLS):
                            nc.sync.dma_start(
                                out=mtens.rearrange("(a p) d -> p a d", p=P),
                                in_=bass.AP(tensor=zt[:].tensor,
                                            offset=zt[:].offset,
                                            ap=[list(zt[:].ap[0]), [0, nblk],
                                                [1, w]]))
                        nc.sync.dma_start(
                            out=h2_full[c.N:NPAD, :].rearrange(
                                "(a p) d -> p a d", p=P),
                            in_=bass.AP(tensor=zt[:].tensor,
                                        offset=zt[:].offset,
                                        ap=[list(zt[:].ap[0]), [0, 2],
                                            [1, c.D]]))
                    wq_t = ap2.tile([P, c.DC, P], BF16, tag="w_t", bufs=3)
                    wk_t = ap2.tile([P, c.DC, P], BF16, tag="w_t", bufs=3)
                    wv_t = ap2.tile([P, c.DC, P], BF16, tag="w_t", bufs=3)
                    nc.sync.dma_start(out=wq_t[:], in_=wq[p_, :, :, :])
                    nc.sync.dma_start(out=wk_t[:], in_=wk[p_, :, :, :])
                    nc.sync.dma_start(out=wv_t[:], in_=wv[p_, :, :, :])

                    # qT for this pair: [128(2 heads x 64), T] (all queries)
                    qT = ap_.tile([P, c.T], BF16, tag="qT", bufs=2)
                    for ns, nn in _nslices(c.T):
                        ps_q = aps.tile([P, 512], FP32, tag="pb")
                        for dc in range(c.DC):
                            nc.tensor.matmul(ps_q[:, :nn], wq_t[:, dc, :],
                                             hT[:, dc, ns:ns + nn],
                                             start=(dc == 0),
                                             stop=(dc == c.DC - 1))
                        nc.vector.tensor_copy(out=qT[:, ns:ns + nn],
                                              in_=ps_q[:, :nn])
                    # kT: [128, T]
                    kT = ap_.tile([P, c.T], BF16, tag="kT", bufs=2)
                    for ns, nn in _nslices(c.T):
                        ps_k = aps.tile([P, 512], FP32, tag="pc")
                        for dc in range(c.DC):
                            nc.tensor.matmul(ps_k[:, :nn], wk_t[:, dc, :],
                                             hT[:, dc, ns:ns + nn],
                                             start=(dc == 0),
                                             stop=(dc == c.DC - 1))
                        nc.vector.tensor_copy(out=kT[:, ns:ns + nn],
                                              in_=ps_k[:, :nn])
                    # v rows + ones col: v_aug [128, KT, 2, 66]
                    v_aug = ap_.tile([P, c.KT, 2, 66], BF16, tag="v_aug",
                                     bufs=2)
                    nc.vector.memset(v_aug[:, :, :, 64:65], 1.0)
                    for kt in range(c.KT):
                        ps_v = aps.tile([P, P], FP32, tag="pb")
                        ksl = slice(kt * P, (kt + 1) * P)
                        for dc in range(c.DC):
                            nc.tensor.matmul(ps_v[:], hT[:, dc, ksl],
                                             wv_t[:, dc, :],
                                             start=(dc == 0),
                                             stop=(dc == c.DC - 1))
                        nc.vector.tensor_copy(
                            out=v_aug[:, kt, :, 0:64],
                            in_=ps_v[:].rearrange("p (h e) -> p h e", h=2))

                    # causal scores -> (mask on diag blocks) -> exp -> AV,
                    # per 512-query slice qs: only key tiles kt < 4*(qs+1).
                    # Scores for the two heads run as concurrent row-group
                    # matmuls into the two banks of one [128,1024] psum
                    # tile; one 1024-wide exp serves both heads.
                    for qs in range(4):
                        qsl = slice(qs * 512, (qs + 1) * 512)
                        nkt = 4 * (qs + 1)
                        ps_av0 = aps.tile([65, 512], FP32, tag="pb")
                        ps_av1 = aps.tile([65, 512], FP32, tag="pc")
                        ps_avs = (ps_av0, ps_av1)
                        for kt in range(nkt):
                            diag = kt >= 4 * qs
                            ps_s = aps.tile([P, 1024], FP32, tag="pa")
                            for h2 in range(2):
                                hsl = slice(h2 * 64, (h2 + 1) * 64)
                                nc.tensor.matmul(
                                    ps_s[:, h2 * 512:(h2 + 1) * 512],
                                    kT[hsl, kt * P:(kt + 1) * P],
                                    qT[hsl, qsl], start=True, stop=True)
                            et = ap3.tile([P, 1024], BF16, tag="et", bufs=4)
                            nc.scalar.activation(out=et[:], in_=ps_s[:],
                                                 func=AF.Exp)
                            if diag:
                                for h2 in range(2):
                                    nc.vector.tensor_tensor(
                                        out=et[:, h2 * 512:(h2 + 1) * 512],
                                        in0=et[:, h2 * 512:(h2 + 1) * 512],
                                        in1=mk_sb[:, kt - 4 * qs, :],
                                        op=AluOpType.mult)
                            for h2 in range(2):
                                nc.tensor.matmul(
                                    ps_avs[h2][:],
                                    v_aug[:, kt, h2, 0:65],
                                    et[:, h2 * 512:(h2 + 1) * 512],
                                    start=(kt == 0), stop=(kt == nkt - 1))
                        # normalize; head 2p -> catT rows 0:64 direct,
                        # head 2p+1 -> rows 64:128 via DMA partition shift
                        for h2 in range(2):
                            rec = ap2.tile([1, 512], FP32, tag="rec", bufs=1)
                            nc.vector.reciprocal(out=rec[:],
                                                 in_=ps_avs[h2][64:65, :])
                            rec_bc = ap2.tile([64, 512], FP32, tag="rec_bc",
                                              bufs=1)
                            nc.gpsimd.partition_broadcast(out_ap=rec_bc[:],
                                                          in_ap=rec[:])
                            if h2 == 0:
                                nc.vector.tensor_tensor(
                                    out=catT[0:64, p_, qsl],
                                    in0=ps_avs[0][0:64, :],
                                    in1=rec_bc[:], op=AluOpType.mult)
                            else:
                                shf = ap2.tile([64, 512], BF16, tag="shf",
                                               bufs=1)
                                nc.vector.tensor_tensor(
                                    out=shf[:], in0=ps_avs[1][0:64, :],
                                    in1=rec_bc[:], op=AluOpType.mult)
                                nc.sync.dma_start(out=catT[64:128, p_, qsl],
                                                  in_=shf[:])

                    # stage this pair's catT into the AllToAll input: my
                    # quarter data goes to BOTH chunk slots j and j+4.
                    for j2 in range(2):
                        half = a2a_in[j2 * 4 * 2 * P:(j2 + 1) * 4 * 2 * P, :]
                        dst = half.rearrange("(q j p) t -> j p q t",
                                             q=4, j=2, p=P)[p_]
                        nc.sync.dma_start(out=dst, in_=catT[:, p_, :])

                # per-pair tiles + hT die here; the tail pool reuses the
                # space (the A2A covers the transition)
                attn_a.__exit__(None, None, None)
                attn_b = tc.tile_pool(name="attn_b", bufs=1)
                ap_ = attn_b.__enter__()

                # prefetch the residual x.T while the AllToAll runs
                xq_sb = ap_.tile([P, c.DC, c.TL], FP32, tag="xq_sb")
                for dc in range(c.DC):
                    nc.sync.dma_start(out=xq_sb[:, dc, :],
                                      in_=xqT[dc * P:(dc + 1) * P, :])

                nc.gpsimd.collective_compute(
                    "AllToAll", AluOpType.bypass, replica_groups=RG,
                    ins=[a2a_in[:]], outs=[a2a_out[:]])

                # full local projection over all 16 ci chunks (foreign
                # chunks hit zero weight blocks) for my 512 tokens
                cat_m = ap_.tile([P, 2 * c.DC, 512], BF16, tag="cat_m")
                for q4 in range(4):
                    nc.sync.dma_start(
                        out=cat_m[:, 4 * q4:4 * q4 + 4, :],
                        in_=a2a_out[q4 * 4 * P:(q4 + 1) * 4 * P, :].rearrange(
                            "(a p) t -> p a t", p=P))

                x2T = ap_.tile([P, c.DC, c.TL], FP32)
                for dco in range(c.DC):
                    wp_t = ap2.tile([P, 2 * c.DC, P], BF16, tag="wp_t",
                                    bufs=2)
                    nc.sync.dma_start(out=wp_t[:], in_=wpT[dco, :, :, :])
                    ps_p = aps.tile([P, 512], FP32, tag="pa")
                    for ci in range(2 * c.DC):
                        nc.tensor.matmul(ps_p[:], wp_t[:, ci, :],
                                         cat_m[:, ci, :],
                                         start=(ci == 0),
                                         stop=(ci == 2 * c.DC - 1))
                    t = ap2.tile([P, 512], FP32, tag="x2t", bufs=2)
                    nc.vector.tensor_scalar(
                        out=t[:], in0=ps_p[:],
                        scalar1=bproj_sb[:, dco:dco + 1], scalar2=None,
                        op0=AluOpType.add)
                    nc.vector.tensor_tensor(
                        out=x2T[:, dco, :], in0=t[:],
                        in1=xq_sb[:, dco, :], op=AluOpType.add)

                # ---- LN2 (one-pass stats on bf16 casts, out-of-place) ----
                # x2n keeps the fp32 normalized value (x2-mu)*r so the gate
                # logits can be computed in fp32 (ln2_g folded into gwT).
                h2T = ap_.tile([P, c.DC, c.TL], BF16)
                x2n = ap_.tile([P, c.DC, c.TL], FP32)
                st2_s = ap2.tile([1, c.TL], FP32, tag="ln2_ss", bufs=1)
                st2_q = ap2.tile([1, c.TL], FP32, tag="ln2_sq", bufs=1)
                ps_sum = aps.tile([1, 512], FP32, tag="pb")
                ps_sq = aps.tile([1, 512], FP32, tag="pc")
                for dc in range(c.DC):
                    xb = ap2.tile([P, 512], BF16, tag="ln2_xb", bufs=3)
                    nc.vector.tensor_copy(out=xb[:], in_=x2T[:, dc, :])
                    nc.tensor.matmul(ps_sum[:], ones_bf[:], xb[:],
                                     start=(dc == 0), stop=(dc == c.DC - 1))
                    sq = ap2.tile([P, 512], BF16, tag="ln2_sqt", bufs=3)
                    nc.vector.tensor_tensor(out=sq[:], in0=xb[:], in1=xb[:],
                                            op=AluOpType.mult)
                    nc.tensor.matmul(ps_sq[:], ones_bf[:], sq[:],
                                     start=(dc == 0), stop=(dc == c.DC - 1))
                nc.vector.tensor_copy(out=st2_s[:], in_=ps_sum[:])
                nc.vector.tensor_copy(out=st2_q[:], in_=ps_sq[:])
                mu2_bf, r2_bf = ln_stats_finish(st2_s, st2_q, c.TL, ap2)
                mubc = ap2.tile([P, 512], BF16, tag="ln_mubc")
                rbc = ap2.tile([P, 512], BF16, tag="ln_rbc")
                nc.gpsimd.partition_broadcast(out_ap=mubc[:], in_ap=mu2_bf[:])
                nc.gpsimd.partition_broadcast(out_ap=rbc[:], in_ap=r2_bf[:])
                for dc in range(c.DC):
                    nc.vector.tensor_tensor(out=x2n[:, dc, :],
                                            in0=x2T[:, dc, :], in1=mubc[:],
                                            op=AluOpType.subtract)
                    nc.vector.tensor_tensor(out=x2n[:, dc, :],
                                            in0=x2n[:, dc, :], in1=rbc[:],
                                            op=AluOpType.mult)
                    nc.scalar.activation(out=h2T[:, dc, :], in_=x2n[:, dc, :],
                                         func=AF.Identity,
                                         scale=ln2g_sb[:, dc:dc + 1],
                                         bias=ln2b_sb[:, dc:dc + 1])

                # gate logits for my block (fp32): [128, TLT, E]
                lg_loc = ap_.tile([P, c.TLT, c.E], FP32)
                for tt in range(c.TLT):
                    ps_l = aps.tile([P, c.E], FP32, tag="pb")
                    tsl = slice(tt * P, (tt + 1) * P)
                    for dc in range(c.DC):
                        nc.tensor.matmul(ps_l[:], x2n[:, dc, tsl],
                                         gwT_sb[:, dc, :],
                                         start=(dc == 0), stop=(dc == c.DC - 1))
                    nc.vector.tensor_tensor(out=lg_loc[:, tt, :], in0=ps_l[:],
                                            in1=gconst_sb[:], op=AluOpType.add)

                # ---- owner-side top-2 routing for my 512 tokens ----
                # m1 = max_e, m2 = 2nd max, w1 = sigmoid(m1-m2), w2 = 1-w1
                rp1 = ap_  # reuse attn pool for small tiles
                m1 = rp1.tile([P, c.TLT], FP32, tag="rt_m1")
                nc.vector.tensor_reduce(out=m1[:], in_=lg_loc[:],
                                        axis=mybir.AxisListType.X,
                                        op=AluOpType.max)
                eq1 = rp1.tile([P, c.TLT, c.E], FP32, tag="rt_eq1")
                for tt in range(c.TLT):
                    nc.vector.tensor_scalar(out=eq1[:, tt, :],
                                            in0=lg_loc[:, tt, :],
                                            scalar1=m1[:, tt:tt + 1],
                                            scalar2=None,
                                            op0=AluOpType.is_equal)
                tmp = rp1.tile([P, c.TLT, c.E], FP32, tag="rt_tmp")
                nc.vector.tensor_scalar(out=tmp[:], in0=eq1[:],
                                        scalar1=-1e30, scalar2=None,
                                        op0=AluOpType.mult)
                nc.vector.tensor_tensor(out=tmp[:], in0=lg_loc[:], in1=tmp[:],
                                        op=AluOpType.add)
                m2 = rp1.tile([P, c.TLT], FP32, tag="rt_m2")
                nc.vector.tensor_reduce(out=m2[:], in_=tmp[:],
                                        axis=mybir.AxisListType.X,
                                        op=AluOpType.max)
                d12 = rp1.tile([P, c.TLT], FP32, tag="rt_d12")
                nc.vector.tensor_tensor(out=d12[:], in0=m1[:], in1=m2[:],
                                        op=AluOpType.subtract)
                w1q = rp1.tile([P, c.TLT], FP32, tag="rt_w1q")
                nc.scalar.activation(out=w1q[:], in_=d12[:], func=AF.Sigmoid)
                # w1q = w1*2046 ; w2q = (1-w1)*2046 ; dq = w1q - w2q
                # (2046 not 2047 so cwq never carries into the tokid bits
                # even when sigmoid rounds to exactly 1.0 in fp32)
                w2q = rp1.tile([P, c.TLT], FP32, tag="rt_w2q")
                nc.vector.tensor_scalar(out=w2q[:], in0=w1q[:],
                                        scalar1=-2046.0, scalar2=2046.0,
                                        op0=AluOpType.mult,
                                        op1=AluOpType.add)
                nc.vector.tensor_scalar(out=w1q[:], in0=w1q[:],
                                        scalar1=2046.0, scalar2=None,
                                        op0=AluOpType.mult)
                dq = rp1.tile([P, c.TLT], FP32, tag="rt_dq")
                nc.vector.tensor_tensor(out=dq[:], in0=w1q[:], in1=w2q[:],
                                        op=AluOpType.subtract)
                # vals[:, tt, e] = flag_e * (tok2048 + 1 + cwq_e) - 1
                vals = rp1.tile([P, c.TLT, c.E], FP32, tag="rt_vals")
                flg = rp1.tile([P, c.TLT, c.E], FP32, tag="rt_flg")
                for tt in range(c.TLT):
                    # cwq = eq1*dq + w2q  (into vals)
                    nc.vector.tensor_scalar(out=vals[:, tt, :],
                                            in0=eq1[:, tt, :],
                                            scalar1=dq[:, tt:tt + 1],
                                            scalar2=w2q[:, tt:tt + 1],
                                            op0=AluOpType.mult,
                                            op1=AluOpType.add)
                    # += tok2048 + 1
                    nc.vector.tensor_scalar(out=vals[:, tt, :],
                                            in0=vals[:, tt, :],
                                            scalar1=tok_sb[:, tt:tt + 1],
                                            scalar2=None,
                                            op0=AluOpType.add)
                    # flag = lg >= m2
                    nc.vector.tensor_scalar(out=flg[:, tt, :],
                                            in0=lg_loc[:, tt, :],
                                            scalar1=m2[:, tt:tt + 1],
                                            scalar2=None,
                                            op0=AluOpType.is_ge)
                nc.vector.tensor_tensor(out=vals[:], in0=vals[:], in1=flg[:],
                                        op=AluOpType.mult)
                nc.vector.tensor_scalar(out=vals[:], in0=vals[:],
                                        scalar1=-1.0, scalar2=None,
                                        op0=AluOpType.add)
                # ship routing vals; tiny AllGather goes out FIRST so the
                # per-expert decode overlaps the big h2 AllGather
                nc.sync.dma_start(
                    out=rv_in.rearrange("(tt p) e -> p tt e", p=P),
                    in_=vals[:])
                nc.gpsimd.collective_compute(
                    "AllGather", AluOpType.bypass, replica_groups=RG,
                    ins=[rv_in[:]], outs=[rv_full[:]])

                # h2 rows (token-major bf16) for AllGather: PE transpose,
                # staged in SBUF then shipped with a single DMA
                h2r_all = ap_.tile([P, c.TLT, c.D], BF16, tag="h2r_all")
                for tt in range(c.TLT):
                    for dc in range(c.DC):
                        ps_t = aps.tile([P, P], BF16, tag="pc")
                        nc.tensor.transpose(
                            out=ps_t[:],
                            in_=h2T[:, dc, tt * P:(tt + 1) * P],
                            identity=ident_bf[:])
                        nc.vector.tensor_copy(
                            out=h2r_all[:, tt, dc * P:(dc + 1) * P],
                            in_=ps_t[:])
                nc.sync.dma_start(
                    out=ag_in.rearrange("(tt p) d -> p tt d", p=P),
                    in_=h2r_all[:])

                # =========================================================
                # PHASE B: AllGather h2 rows
                # =========================================================
                nc.gpsimd.collective_compute(
                    "AllGather", AluOpType.bypass, replica_groups=RG,
                    ins=[ag_in[:]], outs=[h2_full[0:c.N, :]])

                # x2 rows for the final residual: PE transposes fill the
                # AllGather bubble
                for tt in range(c.TLT):
                    for dco in range(c.DC):
                        ps_t = aps.tile([P, P], FP32, tag="pa")
                        nc.tensor.transpose(
                            out=ps_t[:],
                            in_=x2T[:, dco, tt * P:(tt + 1) * P],
                            identity=ident[:])
                        nc.vector.tensor_copy(
                            out=x2r_all[:, tt, dco * P:(dco + 1) * P],
                            in_=ps_t[:])
                attn_b.__exit__(None, None, None)

            # =========================================================
            # PHASE C: post-AG routing: select my expert column, compact,
            # decode (overlaps the big AllGather; only depends on the tiny
            # routing AllGather).  Also the w2 prefetch.
            # =========================================================
            fin_pool = tc.tile_pool(name="fin", bufs=2)
            fp = fin_pool.__enter__()
            moe_pp = tc.tile_pool(name="moe_pp", bufs=1)
            mp1 = moe_pp.__enter__()
            w2_sb = mp1.tile([P, c.FT, c.D], BF16)
            nc.sync.dma_start(out=w2_sb[:], in_=w2.rearrange(
                "(o p) d -> p o d", p=P))
            b2_sb = mp1.tile([P, c.D], FP32)
            nc.sync.dma_start(out=b2_sb[:], in_=bass.AP(
                tensor=b2.tensor, offset=b2.offset,
                ap=[[0, P]] + list(b2.ap)))
            with tc.tile_pool(name="route", bufs=1) as rp:
                # my expert's routing vals: select over the 8 fp32 columns
                lgsel = rp.tile([16, NT16, c.E], FP32)
                nc.sync.dma_start(out=lgsel[:], in_=rv_full.rearrange(
                    "(a p) e -> p a e", p=16))
                # val = sum_e val_e * onehot_e  (others contribute 0)
                sgin = rp.tile([16, NT16 + CAP16], FP32)
                nc.vector.memset(sgin[:, NT16:], SENT)
                ohap = onehot_sb[0:16, :]
                ohbc = bass.AP(tensor=ohap.tensor, offset=ohap.offset,
                               ap=[list(ohap.ap[0]), [0, NT16], [1, c.E]])
                nc.vector.tensor_tensor(out=lgsel[:], in0=lgsel[:],
                                        in1=ohbc, op=AluOpType.mult)
                nc.vector.tensor_reduce(out=sgin[:, 0:NT16], in_=lgsel[:],
                                        axis=mybir.AxisListType.X,
                                        op=AluOpType.add)
                # compact: one sparse_gather over the packed values
                selfull = pp.tile([16, NT16 + CAP16], FP32)
                nf1 = rp.tile([1, 1], mybir.dt.uint32)
                nc.gpsimd.sparse_gather(out=selfull[:], in_=sgin[:],
                                        num_found=nf1[:])
                # decode: v = int(val); tok = v >> 11 ; cw = (v & 2047)/2046
                v32 = pp.tile([16, CAP16], I32)
                nc.vector.tensor_copy(out=v32[:], in_=selfull[:, 0:CAP16])
                t32 = rp.tile([16, CAP16], I32)
                nc.vector.tensor_scalar(out=t32[:], in0=v32[:],
                                        scalar1=11, scalar2=None,
                                        op0=AluOpType.logical_shift_right)
                idsel16 = rp.tile([16, CAP16], I16)
                nc.vector.tensor_copy(out=idsel16[:], in_=t32[:])
                # idx: replicate to the 8 gpsimd core groups (SBUF->SBUF,
                # no DRAM round-trip; the 8 DMAs run concurrently).
                # Emitted before the cw decode so the expert gather isn't
                # queued behind the cw DMAs on the sync engine.
                for g in range(8):
                    nc.sync.dma_start(out=idx_w[g * 16:(g + 1) * 16, :],
                                      in_=idsel16[:])
                cw32 = pp.tile([16, CAP16], I32)
                nc.vector.tensor_scalar(out=cw32[:], in0=v32[:],
                                        scalar1=2047, scalar2=None,
                                        op0=AluOpType.bitwise_and)
                cwf = pp.tile([16, CAP16], FP32)
                nc.vector.tensor_copy(out=cwf[:], in_=cw32[:])
                nc.vector.tensor_scalar(out=cwf[:], in0=cwf[:],
                                        scalar1=1.0 / 2046.0, scalar2=None,
                                        op0=AluOpType.mult)
                # cw: wrapped -> slot-major [128, CI] via DRAM
                nc.sync.dma_start(out=bass.AP(
                    tensor=cwlin.tensor, offset=cwlin.offset,
                    ap=[[1, 16], [16, CAP16]]), in_=cwf[:])
                nc.sync.dma_start(out=cw_all[:], in_=bass.AP(
                    tensor=cwlin.tensor, offset=cwlin.offset,
                    ap=[[1, P], [P, c.CI]]))

            # =========================================================
            # PHASE D: expert FFN over CAP slots in chunks (bf16)
            # =========================================================
            def final_cols(pi):
                # final residual for one 512-column pass
                ns0, w = MOE_COLS[pi]
                for tt in range(c.TLT):
                    ms = fp.tile([P, 512], BF16, tag="ms")
                    nc.sync.dma_start(
                        out=ms[:, :w],
                        in_=moe_s[pi][tt * P:(tt + 1) * P, :])
                    msf = fp.tile([P, 512], FP32, tag="msf")
                    nc.vector.tensor_copy(out=msf[:, :w], in_=ms[:, :w])
                    orow = fp.tile([P, 512], FP32, tag="fout")
                    nc.vector.tensor_tensor(
                        out=orow[:, :w], in0=x2r_all[:, tt, ns0:ns0 + w],
                        in1=msf[:, :w], op=AluOpType.add)
                    nc.sync.dma_start(
                        out=out[tt * P:(tt + 1) * P, ns0:ns0 + w],
                        in_=orow[:, :w])

            with tc.tile_pool(name="moe2", bufs=2) as mp2, \
                 tc.tile_pool(name="moe_ps", bufs=2, space="PSUM") as mps:
                MC16 = c.MOE_CHUNK // 16
                hidTs = []

                def ffn2_cols(mc, pi):
                    # FFN2 for one (chunk, D-column pass) + bias + cw
                    # scale, then one scatter-add
                    csl = slice(mc * MC16, (mc + 1) * MC16)
                    ns0, w = MOE_COLS[pi]
                    orow = mp2.tile([P, c.MCT, w], BF16,
                                    tag=f"orow{pi}", bufs=1)
                    for mt in range(c.MCT):
                        slotcol = mc * c.MCT + mt
                        ps_o = mps.tile([P, 512], FP32, tag="ps_o")
                        for fc in range(c.FT):
                            nc.tensor.matmul(
                                ps_o[:, :w],
                                hidTs[mc][:, fc, mt * P:(mt + 1) * P],
                                w2_sb[:, fc, ns0:ns0 + w],
                                start=(fc == 0), stop=(fc == c.FT - 1))
                        t = mp2.tile([P, 512], FP32, tag="ot")
                        nc.vector.tensor_tensor(
                            out=t[:, :w], in0=ps_o[:, :w],
                            in1=b2_sb[:, ns0:ns0 + w],
                            op=AluOpType.add)
                        nc.vector.tensor_scalar(
                            out=orow[:, mt, :], in0=t[:, :w],
                            scalar1=cw_all[:, slotcol:slotcol + 1],
                            scalar2=None, op0=AluOpType.mult)
                    nc.gpsimd.dma_scatter_add(
                        out_ap=moe_acc[pi][:], in_ap=orow[:],
                        idxs_ap=idx_w[:, csl],
                        num_idxs=c.MOE_CHUNK, num_idxs_reg=c.MOE_CHUNK,
                        elem_size=w)

                # pass 1: per chunk, FFN1 then the LEFT half of FFN2 +
                # scatter.  The left ReduceScatter then runs while the
                # chunks' right-half FFN2 (pass 2) still computes.
                for mc in range(c.MCN):
                    csl = slice(mc * MC16, (mc + 1) * MC16)
                    hsel = mp2.tile([P, c.DC, c.MOE_CHUNK], BF16,
                                    tag=f"hsel{mc}", bufs=1)
                    nc.gpsimd.dma_gather(
                        out_ap=hsel[:], in_ap=h2_full[:, :],
                        idxs_ap=idx_w[:, csl],
                        num_idxs=c.MOE_CHUNK, num_idxs_reg=c.MOE_CHUNK,
                        elem_size=c.D, elem_step=c.D, transpose=True)
                    # FFN1: hidT[ft] = relu(w1[:,ft].T @ hsel + b1[ft])
                    hidT = mp2.tile([P, c.FT, c.MOE_CHUNK], BF16,
                                    tag=f"hidT{mc}", bufs=1)
                    hidTs.append(hidT)
                    for ft in range(c.FT):
                        w1t = mp2.tile([P, c.DC, P], BF16, tag="w1t", bufs=3)
                        nc.sync.dma_start(out=w1t[:], in_=w1[ft, :, :, :])
                        ps_h = mps.tile([P, c.MOE_CHUNK], FP32, tag="ps_h")
                        for dc in range(c.DC):
                            nc.tensor.matmul(ps_h[:],
                                             w1t[:, dc, :],
                                             hsel[:, dc, :],
                                             start=(dc == 0),
                                             stop=(dc == c.DC - 1))
                        nc.scalar.activation(
                            out=hidT[:, ft, :], in_=ps_h[:],
                            func=AF.Relu, bias=b1_sb[:, ft:ft + 1])
                    ffn2_cols(mc, 0)
                nc.gpsimd.collective_compute(
                    "ReduceScatter", AluOpType.add, replica_groups=RG,
                    ins=[moe_acc[0][0:c.N, :]], outs=[moe_s[0][:]])
                # left final residual (overlaps pass 2 compute / RS#R)
                final_cols(0)
                # pass 2 (overlaps RS#L)
                for mc in range(c.MCN):
                    ffn2_cols(mc, 1)
            moe_pp.__exit__(None, None, None)

            # =========================================================
            # PHASE E: last column-pass ReduceScatter + final residual
            # =========================================================
            nc.gpsimd.collective_compute(
                "ReduceScatter", AluOpType.add, replica_groups=RG,
                ins=[moe_acc[1][0:c.N, :]], outs=[moe_s[1][:]])
            final_cols(1)
            fin_pool.__exit__(None, None, None)

    nc.compile()
    return nc


# =====================================================================
# Host side
# =====================================================================

def _rot_table(T, D):
    freqs = (np.arange(0, D, 2, dtype=np.float64) / D)
    t = np.arange(T, dtype=np.float64)
    ang = 2.0 * math.pi * t[:, None] * freqs[None, :]
    rot = np.stack([np.sin(ang), np.cos(ang)], axis=-1).reshape(T, D)
    return rot.astype(np.float32)


def _wtile(w, P_, nI, nO):
    # [nI*128, nO*128] -> [nO, 128, nI, 128]: tile (o) is a contiguous
    # [128p, nI, 128m] block (partition-major rows for single-descriptor DMA)
    return np.ascontiguousarray(
        w.reshape(nI, P_, nO, P_).transpose(2, 1, 0, 3))


def make_in_maps(cfg, x, pos_emb, wq, wk, wv, w_proj, b_proj, ln1_g, ln1_b,
                 ln2_g, ln2_b, gate_w, e_w1, e_b1, e_w2, e_b2):
    import ml_dtypes
    bf16 = ml_dtypes.bfloat16
    c = cfg
    f32 = np.float32
    x = np.asarray(x, f32)
    rot = _rot_table(c.T, c.D)
    pemb = np.asarray(pos_emb, f32)[:c.T]
    scale = c.D ** (-0.5)
    wq_cat = (np.asarray(wq, f32).transpose(1, 0, 2).reshape(c.D, c.D)
              * scale).copy()
    wk_cat = np.asarray(wk, f32).transpose(1, 0, 2).reshape(c.D, c.D).copy()
    wv_cat = np.asarray(wv, f32).transpose(1, 0, 2).reshape(c.D, c.D).copy()
    wpT = np.asarray(w_proj, f32).T.copy()       # [D_in(cat), D_out]
    # gate computed from the normalized x2n: fold ln2_g into the weights,
    # ln2_b contributes a per-expert constant
    ln2_g_ = np.asarray(ln2_g, f32)
    ln2_b_ = np.asarray(ln2_b, f32)
    gw = np.asarray(gate_w, f32)
    gwT = np.ascontiguousarray((gw * ln2_g_[None, :]).T)
    gconst = (gw @ ln2_b_)[None, :].astype(f32)
    posT = np.ascontiguousarray((rot + pemb).T.astype(f32))
    wq_r = _wtile(wq_cat, P, c.DC, c.H // 2).astype(bf16)
    wk_r = _wtile(wk_cat, P, c.DC, c.H // 2).astype(bf16)
    wv_r = _wtile(wv_cat, P, c.DC, c.H // 2).astype(bf16)

    # diag-block causal mask: key (kl*128+r) visible to query q iff
    # kl*128+r <= q  (same pattern for every 512-query slice)
    kk = np.arange(512)
    maskD = np.where(kk[:, None] <= kk[None, :], 1.0, 0.0).astype(bf16)

    in_maps = []
    for core in range(NCORE):
        b, g = core // (NCORE // c.B), core % (NCORE // c.B)
        t0 = g * c.TL
        onehot = np.zeros((1, c.E), f32)
        onehot[0, core % c.E] = 1.0
        xposT = (x[b].T + posT).astype(bf16)
        gtok = (np.arange(c.TL) + core * c.TL).astype(np.float64)
        # per-core w_proj.T pack over 16 ci chunks: ci chunk cchunk =
        # a2a_out rows [cchunk*128, ..): rank i=cchunk//2, pair p_=cchunk%2
        # -> cat feature rows 256*(i%4) + 128*p_.  Foreign group -> zeros.
        W16 = np.zeros((c.DC, P, 2 * c.DC, P), f32)
        for cchunk in range(2 * c.DC):
            i = cchunk // 2
            if i // 4 != core // 4:
                continue
            feat = 256 * (i % 4) + 128 * (cchunk % 2)
            for o in range(c.DC):
                W16[o, :, cchunk, :] = wpT[feat:feat + P,
                                           o * P:(o + 1) * P]
        m = {
            "xposT": np.ascontiguousarray(xposT),
            "xqT": x[b, t0:t0 + c.TL].T.copy(),
            "wq": np.ascontiguousarray(wq_r[2 * g:2 * g + 2]),
            "wk": np.ascontiguousarray(wk_r[2 * g:2 * g + 2]),
            "wv": np.ascontiguousarray(wv_r[2 * g:2 * g + 2]),
            "wpT": W16.astype(bf16),
            "bproj": np.asarray(b_proj, f32),
            "ln1g": np.asarray(ln1_g, f32), "ln1b": np.asarray(ln1_b, f32),
            "ln2g": np.asarray(ln2_g, f32), "ln2b": np.asarray(ln2_b, f32),
            "gwT": gwT,
            "w1": _wtile(np.asarray(e_w1, f32)[core % c.E], P, c.DC,
                         c.FT).astype(bf16),
            "b1": np.asarray(e_b1, f32)[core % c.E].copy(),
            "w2": np.ascontiguousarray(
                np.asarray(e_w2, f32)[core % c.E]).astype(bf16),
            "b2": np.asarray(e_b2, f32)[core % c.E].copy(),
            "maskD": maskD,
            "onehot": onehot,
            "gconst": gconst,
            "tok2048": (gtok * 2048.0 + 1.0).astype(f32),
        }
        in_maps.append(m)
    return in_maps


_CACHE = {}
LAST_RESULTS = None


def _ensure_ntff_hook():
    """Inject antenv.axon_hooks (missing from this image) and install the
    ctypes NTFF profile hook against libaxon_pjrt.so so that
    run_bass_kernel_spmd(trace=True) can capture device profiles."""
    import contextlib
    import ctypes
    import types

    try:
        from antenv.axon_hooks import get_axon_ntff_profile_hook  # noqa: F401
        return True
    except ImportError:
        pass
    so_path = "/opt/axon/libaxon_pjrt.so"
    if not os.path.exists(so_path):
        return False
    lib = ctypes.CDLL(so_path)
    if not hasattr(lib, "axon_start_nrt_profile"):
        return False
    lib.axon_start_nrt_profile.argtypes = [ctypes.POINTER(ctypes.c_int64),
                                           ctypes.c_size_t]
    lib.axon_start_nrt_profile.restype = ctypes.c_int64
    lib.axon_stop_nrt_profile.argtypes = [ctypes.c_char_p]
    lib.axon_stop_nrt_profile.restype = ctypes.c_int64

    @contextlib.contextmanager
    def _hook(output_dir, device_ids):
        import jax
        jax.devices()
        if device_ids:
            ids = (ctypes.c_int64 * len(device_ids))(*device_ids)
            rc = lib.axon_start_nrt_profile(ids, len(device_ids))
        else:
            rc = lib.axon_start_nrt_profile(None, 0)
        if rc != 0:
            raise RuntimeError(f"axon_start_nrt_profile rc={rc}")
        try:
            yield
        finally:
            n = lib.axon_stop_nrt_profile(str(output_dir).encode())
            print(f"ntff profile: {n} file(s) -> {output_dir}",
                  file=sys.stderr)

    mod = types.ModuleType("antenv.axon_hooks")
    state = {"h": _hook}
    mod.set_axon_ntff_profile_hook = lambda h: state.__setitem__("h", h)
    mod.get_axon_ntff_profile_hook = lambda: state["h"]
    sys.modules["antenv.axon_hooks"] = mod
    import antenv
    antenv.axon_hooks = mod
    # avoid remote artifact upload in this container
    from concourse import bass_utils as _bu
    _bu.upload_artifacts = lambda tmpdir: tmpdir
    return True


def kernel(**inputs):
    """Full inputs in (as reference.setup_inputs), full output out."""
    cfg = Cfg()
    key = "full"
    if key not in _CACHE:
        _CACHE[key] = build_nc(cfg)
    nc = _CACHE[key]
    in_maps = make_in_maps(cfg, **{k: np.asarray(v) for k, v in inputs.items()})
    trace = bool(os.environ.get("KB_TRACE"))
    if trace:
        trace = _ensure_ntff_hook()
    from concourse.bass_utils import run_bass_kernel_spmd
    global LAST_RESULTS
    res = run_bass_kernel_spmd(nc, in_maps, list(range(NCORE)), trace=trace)
    LAST_RESULTS = res
    outs = [res.results[i]["out"] for i in range(NCORE)]
    c = cfg
    out = np.zeros((c.B, c.T, c.D), np.float32)
    for core in range(NCORE):
        b, j = core // (NCORE // c.B), core % (NCORE // c.B)
        out[b, j * c.TL:(j + 1) * c.TL] = outs[core]
    return out


# revision 26
# speedup vs baseline: 1.2433x; 1.0342x over previous
"""Trainium2 Bass kernel for nn_Block (moe_routing): transformer block =
LN1 + rotary/pos + 16-head causal attention + residual, then LN2 +
top-2-of-8-expert MoE FFN + residual.

Sharding over 8 NeuronCores:
  - attention: head-group sharded. Core c handles batch b=c//4 and head
    pairs {2g, 2g+1} with g=c%4, over ALL T queries of its batch, with
    static causal block skipping.  Head partials are combined with a
    single 8-core AllToAll of catT quarters (every core writes its
    quarter data into BOTH batch chunk slots; the receiving core's
    per-core projection weights are zero-padded for the foreign group's
    chunks, so the full w_proj contraction over 16 128-chunks yields
    exactly its own batch group's head sum).  Each core then does the
    FULL output projection locally for its 512 owned tokens.
  - experts: core c owns expert c (expert-parallel MoE, CAP=1152 slots).

MoE routing is owner-side: each core computes top-2 gating for its own
512 tokens, packs per expert e a single fp32 value
val_e = flag * (tokid*2048 + cw*2046 + 1) - 1, and AllGathers just the
[N, 8] routing values in a tiny collective issued BEFORE the big h2-row
AllGather.  The whole per-expert routing decode (select column, one
sparse_gather, integer decode) then overlaps the big AllGather, as do
the x2-row transposes for the final residual and the w2 prefetch.

All device activations are kept transposed ([D(part), tokens(free)]) so
every matmul contracts over the partition axis.
"""

import math
import os
import sys

import numpy as np

sys.path.insert(0, "/opt/trn_rl_repo")

import concourse.bass as bass  # noqa: E402
import concourse.tile as tile  # noqa: E402
from concourse import bacc, mybir  # noqa: E402
from concourse.alu_op_type import AluOpType  # noqa: E402
from concourse.masks import make_identity  # noqa: E402

AF = mybir.ActivationFunctionType
FP32 = mybir.dt.float32
BF16 = mybir.dt.bfloat16
I32 = mybir.dt.int32
I16 = mybir.dt.int16
P = 128
NCORE = 8
EPS = 1e-5


class Cfg:
    def __init__(self, T=2048, D=1024, H=16, F=4096, CAP=1152, MOE_CHUNK=384):
        self.B = 2
        self.T = T
        self.D = D
        self.H = H
        self.HD = D // H
        self.F = F
        self.E = 8
        self.CAP = CAP
        self.N = self.B * T            # total tokens
        self.TL = self.N // NCORE      # tokens per core
        self.DC = D // P               # D chunks
        self.KT = T // P               # key tiles
        self.TLT = self.TL // P        # local token tiles
        self.FT = F // P               # F tiles
        self.CI = CAP // P             # capacity tiles
        self.MOE_CHUNK = MOE_CHUNK     # slots per MoE token chunk
        self.MCN = CAP // MOE_CHUNK    # number of MoE chunks
        self.MCT = MOE_CHUNK // P      # 128-tiles per MoE chunk
        assert self.HD == 64 and H % 2 == 0 and self.E == 8
        assert T % 512 == 0 and D % P == 0 and F % P == 0
        assert CAP % MOE_CHUNK == 0 and MOE_CHUNK % P == 0
        assert self.TL % P == 0 and self.N % 16 == 0


def _nslices(n, step=512):
    return [(i, min(step, n - i)) for i in range(0, n, step)]


def build_nc(cfg: Cfg):
    """Build the SPMD Bass program (same program on all 8 cores)."""
    c = cfg
    nc = bacc.Bacc("TRN2", target_bir_lowering=False, debug=False,
                   num_devices=NCORE)
    RG = [list(range(NCORE))]

    # ---------------- I/O ----------------
    def din(name, shape, dt=FP32):
        return nc.dram_tensor(name, list(shape), dt, kind="ExternalInput").ap()

    LP = 2                                      # local head pairs per core
    xposT = din("xposT", (c.D, c.T), BF16)      # (x+pos).T, bf16, full batch
    xqT = din("xqT", (c.D, c.TL))               # x.T my block (residual)
    wq = din("wq", (LP, P, c.DC, P), BF16)      # my 2 pairs, pre-scaled
    wk = din("wk", (LP, P, c.DC, P), BF16)
    wv = din("wv", (LP, P, c.DC, P), BF16)
    # full w_proj.T, packed per (do, ci16): ci chunks of the foreign batch
    # group are ZERO so the 16-chunk contraction of a2a_out picks exactly
    # this core's batch-group head sum.
    wpT = din("wpT", (c.DC, P, 2 * c.DC, P), BF16)
    bproj = din("bproj", (c.D,))
    ln1g = din("ln1g", (c.D,))
    ln1b = din("ln1b", (c.D,))
    ln2g = din("ln2g", (c.D,))
    ln2b = din("ln2b", (c.D,))
    gwT = din("gwT", (c.D, c.E))
    w1 = din("w1", (c.FT, P, c.DC, P), BF16)
    b1 = din("b1", (c.F,))
    w2 = din("w2", (c.F, c.D), BF16)
    b2 = din("b2", (c.D,))
    maskD = din("maskD", (512, 512), BF16)    # diag-block mask {0,1}
    onehot = din("onehot", (1, c.E))
    gconst = din("gconst", (1, c.E))          # gate bias: ln2_b @ gate_w.T
    tok2048 = din("tok2048", (c.TL,))         # (global tokid)*2048 + 1
    out = nc.dram_tensor("out", [c.TL, c.D], FP32, kind="ExternalOutput").ap()

    # ---------------- internal DRAM ----------------
    NPAD = c.N + 2 * P                      # pad rows for sentinel slots
    # Per-pair AllToAll: chunk j (128 rows) at core g = core g's catT
    # columns of pair p_ for batch-local quarter (j%4).  Every core
    # writes both j and j+4; pair 0's A2A runs during pair-1 compute.
    a2a_in = [nc.dram_tensor(f"a2a_in{j}", [NCORE * P, 512], BF16).ap()
              for j in range(2)]
    a2a_out = [nc.dram_tensor(f"a2a_out{j}", [NCORE * P, 512], BF16).ap()
               for j in range(2)]
    rv_in = nc.dram_tensor("rv_in", [c.TL, c.E], FP32).ap()
    rv_full = nc.dram_tensor("rv_full", [c.N, c.E], FP32,
                             addr_space="Shared").ap()
    ag_in = nc.dram_tensor("ag_in", [c.TL, c.D], BF16).ap()
    h2_full = nc.dram_tensor("h2_full", [NPAD, c.D], BF16,
                             addr_space="Shared").ap()
    cwlin = nc.dram_tensor("cwlin", [c.CAP], FP32).ap()
    # MoE combine in 2 column passes of 512: RS#L hides under the R-pass
    # compute; RS#R is partly hidden by the left final-residual work.
    MOE_COLS = ((0, 512), (512, 512))
    moe_acc = [nc.dram_tensor(f"moe_acc{i}", [NPAD, w], BF16).ap()
               for i, (_, w) in enumerate(MOE_COLS)]
    moe_s = [nc.dram_tensor(f"moe_s{i}", [c.TL, w], BF16).ap()
             for i, (_, w) in enumerate(MOE_COLS)]

    NT16 = c.N // 16
    CAP16 = c.CAP // 16
    SENT = float(c.N * 2048)   # sentinel routing val: tokid=N, cw=0

    with tile.TileContext(nc) as tc:
        with tc.tile_pool(name="persist", bufs=1) as pp:
            # ---------------- constants ----------------
            ident = pp.tile([P, P], FP32)
            make_identity(nc, ident[:])
            ident_bf = pp.tile([P, P], BF16)
            make_identity(nc, ident_bf[:])
            ones_bf = pp.tile([P, 1], BF16)
            nc.vector.memset(ones_bf[:], 1.0)

            def load_pcol(ap_dram, n):
                # [n*128] dram -> [128, n] sbuf (per-partition scalars)
                t = pp.tile([P, n], FP32, name=ap_dram.tensor.name + "_sb")
                nc.sync.dma_start(out=t[:], in_=ap_dram.rearrange(
                    "(a p) -> p a", p=P))
                return t

            bproj_sb = load_pcol(bproj, c.DC)
            ln1g_sb = load_pcol(ln1g, c.DC)
            ln1b_sb = load_pcol(ln1b, c.DC)
            ln2g_sb = load_pcol(ln2g, c.DC)
            ln2b_sb = load_pcol(ln2b, c.DC)
            b1_sb = load_pcol(b1, c.FT)

            gwT_sb = pp.tile([P, c.DC, c.E], FP32)
            nc.sync.dma_start(out=gwT_sb[:], in_=gwT.rearrange(
                "(a p) e -> p a e", p=P))
            onehot_sb = pp.tile([P, c.E], FP32)
            nc.sync.dma_start(out=onehot_sb[:], in_=bass.AP(
                tensor=onehot.tensor, offset=onehot.offset,
                ap=[[0, P]] + list(onehot.ap[1:])))
            gconst_sb = pp.tile([P, c.E], FP32)
            nc.sync.dma_start(out=gconst_sb[:], in_=bass.AP(
                tensor=gconst.tensor, offset=gconst.offset,
                ap=[[0, P]] + list(gconst.ap[1:])))
            tok_sb = pp.tile([P, c.TLT], FP32)   # tokid*2048 + 1
            nc.sync.dma_start(out=tok_sb[:], in_=tok2048.rearrange(
                "(tt p) -> p tt", p=P))

            x2r_all = pp.tile([P, c.TLT, c.D], FP32)  # x2 rows (for residual)

            # long-lived routing results
            cw_all = pp.tile([P, c.CI], FP32)        # combine weight per slot
            idx_w = pp.tile([P, CAP16], I16)         # wrapped idx, x8

            # =========================================================
            # PHASE A: attention
            # =========================================================
            with tc.tile_pool(name="attn2", bufs=2) as ap2, \
                 tc.tile_pool(name="attn3", bufs=3) as ap3, \
                 tc.tile_pool(name="attn_ps", bufs=2, space="PSUM") as aps:
                # PSUM budget (8 banks): tag pa = 2 x [128,1024] fp32
                # (2 banks each), tags pb/pc = 2 x [128,512] (1 bank each).
                attn_a = tc.tile_pool(name="attn_a", bufs=1)
                ap_ = attn_a.__enter__()

                # ---- LN1 over the full batch, one-pass stats ----
                # xposT loads are emitted FIRST so the stat matmuls can
                # start as soon as tiles land.
                hT = ap_.tile([P, c.DC, c.T], BF16)   # becomes h (in place)
                for dc in range(c.DC):
                    nc.sync.dma_start(
                        out=hT[:, dc, :],
                        in_=xposT[dc * P:(dc + 1) * P, :])

                stat_s = ap_.tile([1, c.T], FP32, tag="ln_ss")
                stat_q = ap_.tile([1, c.T], FP32, tag="ln_sq")
                for ns, nn in _nslices(c.T):
                    ps_sum = aps.tile([1, 512], FP32, tag="pb")
                    ps_sq = aps.tile([1, 512], FP32, tag="pc")
                    for dc in range(c.DC):
                        nc.tensor.matmul(ps_sum[:, :nn], ones_bf[:],
                                         hT[:, dc, ns:ns + nn],
                                         start=(dc == 0), stop=(dc == c.DC - 1))
                        sq = ap2.tile([P, 512], BF16, tag="ln_sqt")
                        nc.vector.tensor_tensor(out=sq[:, :nn],
                                                in0=hT[:, dc, ns:ns + nn],
                                                in1=hT[:, dc, ns:ns + nn],
                                                op=AluOpType.mult)
                        nc.tensor.matmul(ps_sq[:, :nn], ones_bf[:], sq[:, :nn],
                                         start=(dc == 0), stop=(dc == c.DC - 1))
                    nc.vector.tensor_copy(out=stat_s[:, ns:ns + nn],
                                          in_=ps_sum[:, :nn])
                    nc.vector.tensor_copy(out=stat_q[:, ns:ns + nn],
                                          in_=ps_sq[:, :nn])

                def ln_stats_finish(stat_s, stat_q, ntok, sp):
                    """stat_s <- mean (fp32), returns (mu_bf, r_bf) bf16."""
                    nc.vector.tensor_scalar(out=stat_s[:], in0=stat_s[:],
                                            scalar1=1.0 / c.D, scalar2=None,
                                            op0=AluOpType.mult)
                    musq = sp.tile([1, ntok], FP32, tag="ln_musq", bufs=1)
                    nc.vector.tensor_tensor(out=musq[:], in0=stat_s[:],
                                            in1=stat_s[:], op=AluOpType.mult)
                    # var + eps = E[x^2] + eps - mu^2
                    nc.vector.tensor_scalar(out=stat_q[:], in0=stat_q[:],
                                            scalar1=1.0 / c.D, scalar2=EPS,
                                            op0=AluOpType.mult,
                                            op1=AluOpType.add)
                    nc.vector.tensor_tensor(out=stat_q[:], in0=stat_q[:],
                                            in1=musq[:],
                                            op=AluOpType.subtract)
                    nc.scalar.activation(out=stat_q[:], in_=stat_q[:],
                                         func=AF.Sqrt)
                    nc.vector.reciprocal(out=stat_q[:], in_=stat_q[:])
                    mu_bf = sp.tile([1, ntok], BF16, tag="ln_mubf", bufs=1)
                    r_bf = sp.tile([1, ntok], BF16, tag="ln_rbf", bufs=1)
                    nc.vector.tensor_copy(out=mu_bf[:], in_=stat_s[:])
                    nc.vector.tensor_copy(out=r_bf[:], in_=stat_q[:])
                    return mu_bf, r_bf

                mu_bf, r_bf = ln_stats_finish(stat_s, stat_q, c.T, ap_)

                # apply: h = ((x - mu) * r) * g + b, in place on hT (bf16)
                for ns, nn in _nslices(c.T):
                    mubc = ap2.tile([P, 512], BF16, tag="ln_mubc")
                    rbc = ap2.tile([P, 512], BF16, tag="ln_rbc")
                    nc.gpsimd.partition_broadcast(out_ap=mubc[:, :nn],
                                                  in_ap=mu_bf[:, ns:ns + nn])
                    nc.gpsimd.partition_broadcast(out_ap=rbc[:, :nn],
                                                  in_ap=r_bf[:, ns:ns + nn])
                    for dc in range(c.DC):
                        t = ap2.tile([P, 512], BF16, tag="ln_t", bufs=3)
                        nc.vector.tensor_tensor(out=t[:, :nn],
                                                in0=hT[:, dc, ns:ns + nn],
                                                in1=mubc[:, :nn],
                                                op=AluOpType.subtract)
                        nc.vector.tensor_tensor(out=t[:, :nn], in0=t[:, :nn],
                                                in1=rbc[:, :nn],
                                                op=AluOpType.mult)
                        # scalar engine: out = scale*in + bias
                        nc.scalar.activation(out=hT[:, dc, ns:ns + nn],
                                             in_=t[:, :nn], func=AF.Identity,
                                             scale=ln1g_sb[:, dc:dc + 1],
                                             bias=ln1b_sb[:, dc:dc + 1])

                # diag-block causal masks (4 x [128, 512]), loaded once
                mk_sb = ap_.tile([P, 4, 512], BF16, tag="mk")
                nc.sync.dma_start(out=mk_sb[:], in_=maskD.rearrange(
                    "(kl p) q -> p kl q", p=P))

                catT = ap_.tile([P, 2, c.T], BF16)

                for p_ in range(2):
                    if p_ == 1:
                        # zero moe_acc + h2_full pad rows via broadcast DMA
                        # (during the attention main loop; DMA is idle and
                        # pair-0's weight loads have already gone out)
                        zt = ap_.tile([P, c.D], BF16, tag="zt")
                        nc.vector.memset(zt[:], 0.0)
                        nblk = NPAD // P
                        for mtens, (_, w) in zip(moe_acc, MOE_COLS):
                            nc.sync.dma_start(
                                out=mtens.rearrange("(a p) d -> p a d", p=P),
                                in_=bass.AP(tensor=zt[:].tensor,
                                            offset=zt[:].offset,
                                            ap=[list(zt[:].ap[0]), [0, nblk],
                                                [1, w]]))
                        nc.sync.dma_start(
                            out=h2_full[c.N:NPAD, :].rearrange(
                                "(a p) d -> p a d", p=P),
                            in_=bass.AP(tensor=zt[:].tensor,
                                        offset=zt[:].offset,
                                        ap=[list(zt[:].ap[0]), [0, 2],
                                            [1, c.D]]))
                    wq_t = ap2.tile([P, c.DC, P], BF16, tag="w_t", bufs=3)
                    wk_t = ap2.tile([P, c.DC, P], BF16, tag="w_t", bufs=3)
                    wv_t = ap2.tile([P, c.DC, P], BF16, tag="w_t", bufs=3)
                    nc.sync.dma_start(out=wq_t[:], in_=wq[p_, :, :, :])
                    nc.sync.dma_start(out=wk_t[:], in_=wk[p_, :, :, :])
                    nc.sync.dma_start(out=wv_t[:], in_=wv[p_, :, :, :])

                    # qT for this pair: [128(2 heads x 64), T] (all queries)
                    # psum evacuations alternate DVE/ACT to halve the
                    # copy-engine serial time
                    qT = ap_.tile([P, c.T], BF16, tag="qT", bufs=2)
                    for si, (ns, nn) in enumerate(_nslices(c.T)):
                        ps_q = aps.tile([P, 512], FP32, tag="pb")
                        for dc in range(c.DC):
                            nc.tensor.matmul(ps_q[:, :nn], wq_t[:, dc, :],
                                             hT[:, dc, ns:ns + nn],
                                             start=(dc == 0),
                                             stop=(dc == c.DC - 1))
                        if si % 2 == 0:
                            nc.vector.tensor_copy(out=qT[:, ns:ns + nn],
                                                  in_=ps_q[:, :nn])
                        else:
                            nc.scalar.copy(out=qT[:, ns:ns + nn],
                                           in_=ps_q[:, :nn])
                    # kT: [128, T]
                    kT = ap_.tile([P, c.T], BF16, tag="kT", bufs=2)
                    for si, (ns, nn) in enumerate(_nslices(c.T)):
                        ps_k = aps.tile([P, 512], FP32, tag="pc")
                        for dc in range(c.DC):
                            nc.tensor.matmul(ps_k[:, :nn], wk_t[:, dc, :],
                                             hT[:, dc, ns:ns + nn],
                                             start=(dc == 0),
                                             stop=(dc == c.DC - 1))
                        if si % 2 == 0:
                            nc.scalar.copy(out=kT[:, ns:ns + nn],
                                           in_=ps_k[:, :nn])
                        else:
                            nc.vector.tensor_copy(out=kT[:, ns:ns + nn],
                                                  in_=ps_k[:, :nn])
                    # v rows + ones col: v_aug [128, KT, 2, 66]
                    v_aug = ap_.tile([P, c.KT, 2, 66], BF16, tag="v_aug",
                                     bufs=2)
                    nc.vector.memset(v_aug[:, :, :, 64:65], 1.0)
                    for kt in range(c.KT):
                        ps_v = aps.tile([P, P], FP32, tag="pb")
                        ksl = slice(kt * P, (kt + 1) * P)
                        for dc in range(c.DC):
                            nc.tensor.matmul(ps_v[:], hT[:, dc, ksl],
                                             wv_t[:, dc, :],
                                             start=(dc == 0),
                                             stop=(dc == c.DC - 1))
                        if kt % 2 == 0:
                            nc.vector.tensor_copy(
                                out=v_aug[:, kt, :, 0:64],
                                in_=ps_v[:].rearrange("p (h e) -> p h e",
                                                      h=2))
                        else:
                            nc.scalar.copy(
                                out=v_aug[:, kt, :, 0:64],
                                in_=ps_v[:].rearrange("p (h e) -> p h e",
                                                      h=2))

                    # causal scores -> (mask on diag blocks) -> exp -> AV,
                    # per 512-query slice qs: only key tiles kt < 4*(qs+1).
                    # Scores for the two heads run as concurrent row-group
                    # matmuls into the two banks of one [128,1024] psum
                    # tile; one 1024-wide exp serves both heads.
                    for qs in range(4):
                        qsl = slice(qs * 512, (qs + 1) * 512)
                        nkt = 4 * (qs + 1)
                        ps_av0 = aps.tile([65, 512], FP32, tag="pb")
                        ps_av1 = aps.tile([65, 512], FP32, tag="pc")
                        ps_avs = (ps_av0, ps_av1)
                        for kt in range(nkt):
                            diag = kt >= 4 * qs
                            ps_s = aps.tile([P, 1024], FP32, tag="pa")
                            for h2 in range(2):
                                hsl = slice(h2 * 64, (h2 + 1) * 64)
                                nc.tensor.matmul(
                                    ps_s[:, h2 * 512:(h2 + 1) * 512],
                                    kT[hsl, kt * P:(kt + 1) * P],
                                    qT[hsl, qsl], start=True, stop=True)
                            et = ap3.tile([P, 1024], BF16, tag="et", bufs=4)
                            nc.scalar.activation(out=et[:], in_=ps_s[:],
                                                 func=AF.Exp)
                            if diag:
                                for h2 in range(2):
                                    nc.vector.tensor_tensor(
                                        out=et[:, h2 * 512:(h2 + 1) * 512],
                                        in0=et[:, h2 * 512:(h2 + 1) * 512],
                                        in1=mk_sb[:, kt - 4 * qs, :],
                                        op=AluOpType.mult)
                            for h2 in range(2):
                                nc.tensor.matmul(
                                    ps_avs[h2][:],
                                    v_aug[:, kt, h2, 0:65],
                                    et[:, h2 * 512:(h2 + 1) * 512],
                                    start=(kt == 0), stop=(kt == nkt - 1))
                        # normalize; head 2p -> catT rows 0:64 direct,
                        # head 2p+1 -> rows 64:128 via DMA partition shift
                        for h2 in range(2):
                            rec = ap2.tile([1, 512], FP32, tag="rec", bufs=1)
                            nc.vector.reciprocal(out=rec[:],
                                                 in_=ps_avs[h2][64:65, :])
                            rec_bc = ap2.tile([64, 512], FP32, tag="rec_bc",
                                              bufs=1)
                            nc.gpsimd.partition_broadcast(out_ap=rec_bc[:],
                                                          in_ap=rec[:])
                            if h2 == 0:
                                nc.vector.tensor_tensor(
                                    out=catT[0:64, p_, qsl],
                                    in0=ps_avs[0][0:64, :],
                                    in1=rec_bc[:], op=AluOpType.mult)
                            else:
                                shf = ap2.tile([64, 512], BF16, tag="shf",
                                               bufs=1)
                                nc.vector.tensor_tensor(
                                    out=shf[:], in0=ps_avs[1][0:64, :],
                                    in1=rec_bc[:], op=AluOpType.mult)
                                nc.sync.dma_start(out=catT[64:128, p_, qsl],
                                                  in_=shf[:])

                    # stage this pair's catT into its AllToAll input (my
                    # quarter data goes to BOTH chunk slots j and j+4) and
                    # launch the pair's A2A: pair-0's A2A overlaps pair-1
                    # compute.
                    for j2 in range(2):
                        half = a2a_in[p_][j2 * 4 * P:(j2 + 1) * 4 * P, :]
                        dst = half.rearrange("(q p) t -> p q t", q=4, p=P)
                        nc.sync.dma_start(out=dst, in_=catT[:, p_, :])
                    nc.gpsimd.collective_compute(
                        "AllToAll", AluOpType.bypass, replica_groups=RG,
                        ins=[a2a_in[p_][:]], outs=[a2a_out[p_][:]])

                # per-pair tiles + hT die here; the tail pool reuses the
                # space (the A2A covers the transition)
                attn_a.__exit__(None, None, None)
                attn_b = tc.tile_pool(name="attn_b", bufs=1)
                ap_ = attn_b.__enter__()

                # prefetch the residual x.T while the AllToAll runs
                xq_sb = ap_.tile([P, c.DC, c.TL], FP32, tag="xq_sb")
                for dc in range(c.DC):
                    nc.sync.dma_start(out=xq_sb[:, dc, :],
                                      in_=xqT[dc * P:(dc + 1) * P, :])

                # full local projection over all 16 ci chunks (foreign
                # chunks hit zero weight blocks) for my 512 tokens;
                # ci = p_*8 + rank
                cat_m = ap_.tile([P, 2 * c.DC, 512], BF16, tag="cat_m")
                for p_ in range(2):
                    nc.sync.dma_start(
                        out=cat_m[:, p_ * 8:p_ * 8 + 8, :],
                        in_=a2a_out[p_][:].rearrange(
                            "(a p) t -> p a t", p=P))

                x2T = ap_.tile([P, c.DC, c.TL], FP32)
                for dco in range(c.DC):
                    wp_t = ap2.tile([P, 2 * c.DC, P], BF16, tag="wp_t",
                                    bufs=2)
                    nc.sync.dma_start(out=wp_t[:], in_=wpT[dco, :, :, :])
                    ps_p = aps.tile([P, 512], FP32, tag="pa")
                    for ci in range(2 * c.DC):
                        nc.tensor.matmul(ps_p[:], wp_t[:, ci, :],
                                         cat_m[:, ci, :],
                                         start=(ci == 0),
                                         stop=(ci == 2 * c.DC - 1))
                    t = ap2.tile([P, 512], FP32, tag="x2t", bufs=2)
                    nc.vector.tensor_scalar(
                        out=t[:], in0=ps_p[:],
                        scalar1=bproj_sb[:, dco:dco + 1], scalar2=None,
                        op0=AluOpType.add)
                    nc.vector.tensor_tensor(
                        out=x2T[:, dco, :], in0=t[:],
                        in1=xq_sb[:, dco, :], op=AluOpType.add)

                # ---- LN2 (one-pass stats on bf16 casts, out-of-place) ----
                # x2n keeps the fp32 normalized value (x2-mu)*r so the gate
                # logits can be computed in fp32 (ln2_g folded into gwT).
                h2T = ap_.tile([P, c.DC, c.TL], BF16)
                x2n = ap_.tile([P, c.DC, c.TL], FP32)
                st2_s = ap2.tile([1, c.TL], FP32, tag="ln2_ss", bufs=1)
                st2_q = ap2.tile([1, c.TL], FP32, tag="ln2_sq", bufs=1)
                ps_sum = aps.tile([1, 512], FP32, tag="pb")
                ps_sq = aps.tile([1, 512], FP32, tag="pc")
                for dc in range(c.DC):
                    xb = ap2.tile([P, 512], BF16, tag="ln2_xb", bufs=3)
                    nc.vector.tensor_copy(out=xb[:], in_=x2T[:, dc, :])
                    nc.tensor.matmul(ps_sum[:], ones_bf[:], xb[:],
                                     start=(dc == 0), stop=(dc == c.DC - 1))
                    sq = ap2.tile([P, 512], BF16, tag="ln2_sqt", bufs=3)
                    nc.vector.tensor_tensor(out=sq[:], in0=xb[:], in1=xb[:],
                                            op=AluOpType.mult)
                    nc.tensor.matmul(ps_sq[:], ones_bf[:], sq[:],
                                     start=(dc == 0), stop=(dc == c.DC - 1))
                nc.vector.tensor_copy(out=st2_s[:], in_=ps_sum[:])
                nc.vector.tensor_copy(out=st2_q[:], in_=ps_sq[:])
                mu2_bf, r2_bf = ln_stats_finish(st2_s, st2_q, c.TL, ap2)
                mubc = ap2.tile([P, 512], BF16, tag="ln_mubc")
                rbc = ap2.tile([P, 512], BF16, tag="ln_rbc")
                nc.gpsimd.partition_broadcast(out_ap=mubc[:], in_ap=mu2_bf[:])
                nc.gpsimd.partition_broadcast(out_ap=rbc[:], in_ap=r2_bf[:])
                for dc in range(c.DC):
                    nc.vector.tensor_tensor(out=x2n[:, dc, :],
                                            in0=x2T[:, dc, :], in1=mubc[:],
                                            op=AluOpType.subtract)
                    nc.vector.tensor_tensor(out=x2n[:, dc, :],
                                            in0=x2n[:, dc, :], in1=rbc[:],
                                            op=AluOpType.mult)
                    nc.scalar.activation(out=h2T[:, dc, :], in_=x2n[:, dc, :],
                                         func=AF.Identity,
                                         scale=ln2g_sb[:, dc:dc + 1],
                                         bias=ln2b_sb[:, dc:dc + 1])

                # gate logits for my block (fp32): [128, TLT, E]
                lg_loc = ap_.tile([P, c.TLT, c.E], FP32)
                for tt in range(c.TLT):
                    ps_l = aps.tile([P, c.E], FP32, tag="pb")
                    tsl = slice(tt * P, (tt + 1) * P)
                    for dc in range(c.DC):
                        nc.tensor.matmul(ps_l[:], x2n[:, dc, tsl],
                                         gwT_sb[:, dc, :],
                                         start=(dc == 0), stop=(dc == c.DC - 1))
                    nc.vector.tensor_tensor(out=lg_loc[:, tt, :], in0=ps_l[:],
                                            in1=gconst_sb[:], op=AluOpType.add)

                # ---- owner-side top-2 routing for my 512 tokens ----
                # m1 = max_e, m2 = 2nd max, w1 = sigmoid(m1-m2), w2 = 1-w1
                rp1 = ap_  # reuse attn pool for small tiles
                m1 = rp1.tile([P, c.TLT], FP32, tag="rt_m1")
                nc.vector.tensor_reduce(out=m1[:], in_=lg_loc[:],
                                        axis=mybir.AxisListType.X,
                                        op=AluOpType.max)
                eq1 = rp1.tile([P, c.TLT, c.E], FP32, tag="rt_eq1")
                for tt in range(c.TLT):
                    nc.vector.tensor_scalar(out=eq1[:, tt, :],
                                            in0=lg_loc[:, tt, :],
                                            scalar1=m1[:, tt:tt + 1],
                                            scalar2=None,
                                            op0=AluOpType.is_equal)
                tmp = rp1.tile([P, c.TLT, c.E], FP32, tag="rt_tmp")
                nc.vector.tensor_scalar(out=tmp[:], in0=eq1[:],
                                        scalar1=-1e30, scalar2=None,
                                        op0=AluOpType.mult)
                nc.vector.tensor_tensor(out=tmp[:], in0=lg_loc[:], in1=tmp[:],
                                        op=AluOpType.add)
                m2 = rp1.tile([P, c.TLT], FP32, tag="rt_m2")
                nc.vector.tensor_reduce(out=m2[:], in_=tmp[:],
                                        axis=mybir.AxisListType.X,
                                        op=AluOpType.max)
                d12 = rp1.tile([P, c.TLT], FP32, tag="rt_d12")
                nc.vector.tensor_tensor(out=d12[:], in0=m1[:], in1=m2[:],
                                        op=AluOpType.subtract)
                w1q = rp1.tile([P, c.TLT], FP32, tag="rt_w1q")
                nc.scalar.activation(out=w1q[:], in_=d12[:], func=AF.Sigmoid)
                # w1q = w1*2046 ; w2q = (1-w1)*2046 ; dq = w1q - w2q
                # (2046 not 2047 so cwq never carries into the tokid bits
                # even when sigmoid rounds to exactly 1.0 in fp32)
                w2q = rp1.tile([P, c.TLT], FP32, tag="rt_w2q")
                nc.vector.tensor_scalar(out=w2q[:], in0=w1q[:],
                                        scalar1=-2046.0, scalar2=2046.0,
                                        op0=AluOpType.mult,
                                        op1=AluOpType.add)
                nc.vector.tensor_scalar(out=w1q[:], in0=w1q[:],
                                        scalar1=2046.0, scalar2=None,
                                        op0=AluOpType.mult)
                dq = rp1.tile([P, c.TLT], FP32, tag="rt_dq")
                nc.vector.tensor_tensor(out=dq[:], in0=w1q[:], in1=w2q[:],
                                        op=AluOpType.subtract)
                # vals[:, tt, e] = flag_e * (tok2048 + 1 + cwq_e) - 1
                vals = rp1.tile([P, c.TLT, c.E], FP32, tag="rt_vals")
                flg = rp1.tile([P, c.TLT, c.E], FP32, tag="rt_flg")
                for tt in range(c.TLT):
                    # cwq = eq1*dq + w2q  (into vals)
                    nc.vector.tensor_scalar(out=vals[:, tt, :],
                                            in0=eq1[:, tt, :],
                                            scalar1=dq[:, tt:tt + 1],
                                            scalar2=w2q[:, tt:tt + 1],
                                            op0=AluOpType.mult,
                                            op1=AluOpType.add)
                    # += tok2048 + 1
                    nc.vector.tensor_scalar(out=vals[:, tt, :],
                                            in0=vals[:, tt, :],
                                            scalar1=tok_sb[:, tt:tt + 1],
                                            scalar2=None,
                                            op0=AluOpType.add)
                    # flag = lg >= m2
                    nc.vector.tensor_scalar(out=flg[:, tt, :],
                                            in0=lg_loc[:, tt, :],
                                            scalar1=m2[:, tt:tt + 1],
                                            scalar2=None,
                                            op0=AluOpType.is_ge)
                nc.vector.tensor_tensor(out=vals[:], in0=vals[:], in1=flg[:],
                                        op=AluOpType.mult)
                nc.vector.tensor_scalar(out=vals[:], in0=vals[:],
                                        scalar1=-1.0, scalar2=None,
                                        op0=AluOpType.add)
                # ship routing vals; tiny AllGather goes out FIRST so the
                # per-expert decode overlaps the big h2 AllGather
                nc.sync.dma_start(
                    out=rv_in.rearrange("(tt p) e -> p tt e", p=P),
                    in_=vals[:])
                nc.gpsimd.collective_compute(
                    "AllGather", AluOpType.bypass, replica_groups=RG,
                    ins=[rv_in[:]], outs=[rv_full[:]])

                # h2 rows (token-major bf16) for AllGather: PE transpose,
                # staged in SBUF then shipped with a single DMA
                h2r_all = ap_.tile([P, c.TLT, c.D], BF16, tag="h2r_all")
                for tt in range(c.TLT):
                    for dc in range(c.DC):
                        ps_t = aps.tile([P, P], BF16, tag="pc")
                        nc.tensor.transpose(
                            out=ps_t[:],
                            in_=h2T[:, dc, tt * P:(tt + 1) * P],
                            identity=ident_bf[:])
                        nc.vector.tensor_copy(
                            out=h2r_all[:, tt, dc * P:(dc + 1) * P],
                            in_=ps_t[:])
                nc.sync.dma_start(
                    out=ag_in.rearrange("(tt p) d -> p tt d", p=P),
                    in_=h2r_all[:])

                # ---- per-expert routing decode, BEFORE the big AllGather
                # is triggered: collectives and gpsimd ops share one
                # in-order engine stream, so the sparse_gather must be
                # emitted first or it waits for the whole AllGather.
                # Only depends on the tiny routing AllGather.
                with tc.tile_pool(name="route", bufs=1) as rp:
                    lgsel = rp.tile([16, NT16, c.E], FP32)
                    nc.sync.dma_start(out=lgsel[:], in_=rv_full.rearrange(
                        "(a p) e -> p a e", p=16))
                    # val = sum_e val_e * onehot_e  (others contribute 0)
                    sgin = rp.tile([16, NT16 + CAP16], FP32)
                    nc.vector.memset(sgin[:, NT16:], SENT)
                    ohap = onehot_sb[0:16, :]
                    ohbc = bass.AP(tensor=ohap.tensor, offset=ohap.offset,
                                   ap=[list(ohap.ap[0]), [0, NT16], [1, c.E]])
                    nc.vector.tensor_tensor(out=lgsel[:], in0=lgsel[:],
                                            in1=ohbc, op=AluOpType.mult)
                    nc.vector.tensor_reduce(out=sgin[:, 0:NT16],
                                            in_=lgsel[:],
                                            axis=mybir.AxisListType.X,
                                            op=AluOpType.add)
                    # compact: one sparse_gather over the packed values
                    selfull = pp.tile([16, NT16 + CAP16], FP32)
                    nf1 = rp.tile([1, 1], mybir.dt.uint32)
                    nc.gpsimd.sparse_gather(out=selfull[:], in_=sgin[:],
                                            num_found=nf1[:])
                    # decode: v = int(val); tok = v>>11; cw = (v&2047)/2046
                    v32 = pp.tile([16, CAP16], I32)
                    nc.vector.tensor_copy(out=v32[:], in_=selfull[:, 0:CAP16])
                    t32 = rp.tile([16, CAP16], I32)
                    nc.vector.tensor_scalar(out=t32[:], in0=v32[:],
                                            scalar1=11, scalar2=None,
                                            op0=AluOpType.logical_shift_right)
                    idsel16 = rp.tile([16, CAP16], I16)
                    nc.vector.tensor_copy(out=idsel16[:], in_=t32[:])
                    # idx: replicate to the 8 gpsimd core groups
                    for g in range(8):
                        nc.sync.dma_start(out=idx_w[g * 16:(g + 1) * 16, :],
                                          in_=idsel16[:])
                    cw32 = pp.tile([16, CAP16], I32)
                    nc.vector.tensor_scalar(out=cw32[:], in0=v32[:],
                                            scalar1=2047, scalar2=None,
                                            op0=AluOpType.bitwise_and)
                    cwf = pp.tile([16, CAP16], FP32)
                    nc.vector.tensor_copy(out=cwf[:], in_=cw32[:])
                    nc.vector.tensor_scalar(out=cwf[:], in0=cwf[:],
                                            scalar1=1.0 / 2046.0,
                                            scalar2=None,
                                            op0=AluOpType.mult)
                    # cw: wrapped -> slot-major [128, CI] via DRAM
                    nc.sync.dma_start(out=bass.AP(
                        tensor=cwlin.tensor, offset=cwlin.offset,
                        ap=[[1, 16], [16, CAP16]]), in_=cwf[:])
                    nc.sync.dma_start(out=cw_all[:], in_=bass.AP(
                        tensor=cwlin.tensor, offset=cwlin.offset,
                        ap=[[1, P], [P, c.CI]]))

                # =========================================================
                # PHASE B: AllGather h2 rows
                # =========================================================
                nc.gpsimd.collective_compute(
                    "AllGather", AluOpType.bypass, replica_groups=RG,
                    ins=[ag_in[:]], outs=[h2_full[0:c.N, :]])

                # x2 rows for the final residual: PE transposes fill the
                # AllGather bubble
                for tt in range(c.TLT):
                    for dco in range(c.DC):
                        ps_t = aps.tile([P, P], FP32, tag="pa")
                        nc.tensor.transpose(
                            out=ps_t[:],
                            in_=x2T[:, dco, tt * P:(tt + 1) * P],
                            identity=ident[:])
                        nc.vector.tensor_copy(
                            out=x2r_all[:, tt, dco * P:(dco + 1) * P],
                            in_=ps_t[:])
                attn_b.__exit__(None, None, None)

            # =========================================================
            # PHASE C: w2 prefetch (overlaps the big AllGather)
            # =========================================================
            fin_pool = tc.tile_pool(name="fin", bufs=2)
            fp = fin_pool.__enter__()
            moe_pp = tc.tile_pool(name="moe_pp", bufs=1)
            mp1 = moe_pp.__enter__()
            w2_sb = mp1.tile([P, c.FT, c.D], BF16)
            nc.sync.dma_start(out=w2_sb[:], in_=w2.rearrange(
                "(o p) d -> p o d", p=P))
            b2_sb = mp1.tile([P, c.D], FP32)
            nc.sync.dma_start(out=b2_sb[:], in_=bass.AP(
                tensor=b2.tensor, offset=b2.offset,
                ap=[[0, P]] + list(b2.ap)))

            # =========================================================
            # PHASE D: expert FFN over CAP slots in chunks (bf16)
            # =========================================================
            def final_cols(pi):
                # final residual for one 512-column pass
                ns0, w = MOE_COLS[pi]
                for tt in range(c.TLT):
                    ms = fp.tile([P, 512], BF16, tag="ms")
                    nc.sync.dma_start(
                        out=ms[:, :w],
                        in_=moe_s[pi][tt * P:(tt + 1) * P, :])
                    msf = fp.tile([P, 512], FP32, tag="msf")
                    nc.vector.tensor_copy(out=msf[:, :w], in_=ms[:, :w])
                    orow = fp.tile([P, 512], FP32, tag="fout")
                    nc.vector.tensor_tensor(
                        out=orow[:, :w], in0=x2r_all[:, tt, ns0:ns0 + w],
                        in1=msf[:, :w], op=AluOpType.add)
                    nc.sync.dma_start(
                        out=out[tt * P:(tt + 1) * P, ns0:ns0 + w],
                        in_=orow[:, :w])

            with tc.tile_pool(name="moe2", bufs=2) as mp2, \
                 tc.tile_pool(name="moe_ps", bufs=2, space="PSUM") as mps:
                MC16 = c.MOE_CHUNK // 16
                hidTs = []

                def ffn2_cols(mc, pi):
                    # FFN2 for one (chunk, D-column pass) + bias + cw
                    # scale, then one scatter-add.  bufs=2 so pass-2
                    # compute isn't serialized behind the scatters that
                    # queue on gpsimd after the RS#L trigger.
                    csl = slice(mc * MC16, (mc + 1) * MC16)
                    ns0, w = MOE_COLS[pi]
                    orow = mp2.tile([P, c.MCT, w], BF16,
                                    tag=f"orow{pi}", bufs=2)
                    for mt in range(c.MCT):
                        slotcol = mc * c.MCT + mt
                        ps_o = mps.tile([P, 512], FP32, tag="ps_o")
                        for fc in range(c.FT):
                            nc.tensor.matmul(
                                ps_o[:, :w],
                                hidTs[mc][:, fc, mt * P:(mt + 1) * P],
                                w2_sb[:, fc, ns0:ns0 + w],
                                start=(fc == 0), stop=(fc == c.FT - 1))
                        t = mp2.tile([P, 512], FP32, tag="ot")
                        nc.vector.tensor_tensor(
                            out=t[:, :w], in0=ps_o[:, :w],
                            in1=b2_sb[:, ns0:ns0 + w],
                            op=AluOpType.add)
                        nc.vector.tensor_scalar(
                            out=orow[:, mt, :], in0=t[:, :w],
                            scalar1=cw_all[:, slotcol:slotcol + 1],
                            scalar2=None, op0=AluOpType.mult)
                    nc.gpsimd.dma_scatter_add(
                        out_ap=moe_acc[pi][:], in_ap=orow[:],
                        idxs_ap=idx_w[:, csl],
                        num_idxs=c.MOE_CHUNK, num_idxs_reg=c.MOE_CHUNK,
                        elem_size=w)

                # pass 1: per chunk, FFN1 then the LEFT half of FFN2 +
                # scatter.  The left ReduceScatter then runs while the
                # chunks' right-half FFN2 (pass 2) still computes.
                for mc in range(c.MCN):
                    csl = slice(mc * MC16, (mc + 1) * MC16)
                    hsel = mp2.tile([P, c.DC, c.MOE_CHUNK], BF16,
                                    tag="hsel", bufs=2)
                    nc.gpsimd.dma_gather(
                        out_ap=hsel[:], in_ap=h2_full[:, :],
                        idxs_ap=idx_w[:, csl],
                        num_idxs=c.MOE_CHUNK, num_idxs_reg=c.MOE_CHUNK,
                        elem_size=c.D, elem_step=c.D, transpose=True)
                    # FFN1: hidT[ft] = relu(w1[:,ft].T @ hsel + b1[ft])
                    hidT = mp2.tile([P, c.FT, c.MOE_CHUNK], BF16,
                                    tag=f"hidT{mc}", bufs=1)
                    hidTs.append(hidT)
                    for ft in range(c.FT):
                        w1t = mp2.tile([P, c.DC, P], BF16, tag="w1t", bufs=3)
                        nc.sync.dma_start(out=w1t[:], in_=w1[ft, :, :, :])
                        ps_h = mps.tile([P, c.MOE_CHUNK], FP32, tag="ps_h")
                        for dc in range(c.DC):
                            nc.tensor.matmul(ps_h[:],
                                             w1t[:, dc, :],
                                             hsel[:, dc, :],
                                             start=(dc == 0),
                                             stop=(dc == c.DC - 1))
                        nc.scalar.activation(
                            out=hidT[:, ft, :], in_=ps_h[:],
                            func=AF.Relu, bias=b1_sb[:, ft:ft + 1])
                    ffn2_cols(mc, 0)
                nc.gpsimd.collective_compute(
                    "ReduceScatter", AluOpType.add, replica_groups=RG,
                    ins=[moe_acc[0][0:c.N, :]], outs=[moe_s[0][:]])
                # left final residual (overlaps pass 2 compute / RS#R)
                final_cols(0)
                # pass 2 (overlaps RS#L)
                for mc in range(c.MCN):
                    ffn2_cols(mc, 1)
            moe_pp.__exit__(None, None, None)

            # =========================================================
            # PHASE E: last column-pass ReduceScatter + final residual
            # =========================================================
            nc.gpsimd.collective_compute(
                "ReduceScatter", AluOpType.add, replica_groups=RG,
                ins=[moe_acc[1][0:c.N, :]], outs=[moe_s[1][:]])
            final_cols(1)
            fin_pool.__exit__(None, None, None)

    nc.compile()
    return nc


# =====================================================================
# Host side
# =====================================================================

def _rot_table(T, D):
    freqs = (np.arange(0, D, 2, dtype=np.float64) / D)
    t = np.arange(T, dtype=np.float64)
    ang = 2.0 * math.pi * t[:, None] * freqs[None, :]
    rot = np.stack([np.sin(ang), np.cos(ang)], axis=-1).reshape(T, D)
    return rot.astype(np.float32)


def _wtile(w, P_, nI, nO):
    # [nI*128, nO*128] -> [nO, 128, nI, 128]: tile (o) is a contiguous
    # [128p, nI, 128m] block (partition-major rows for single-descriptor DMA)
    return np.ascontiguousarray(
        w.reshape(nI, P_, nO, P_).transpose(2, 1, 0, 3))


def make_in_maps(cfg, x, pos_emb, wq, wk, wv, w_proj, b_proj, ln1_g, ln1_b,
                 ln2_g, ln2_b, gate_w, e_w1, e_b1, e_w2, e_b2):
    import ml_dtypes
    bf16 = ml_dtypes.bfloat16
    c = cfg
    f32 = np.float32
    x = np.asarray(x, f32)
    rot = _rot_table(c.T, c.D)
    pemb = np.asarray(pos_emb, f32)[:c.T]
    scale = c.D ** (-0.5)
    wq_cat = (np.asarray(wq, f32).transpose(1, 0, 2).reshape(c.D, c.D)
              * scale).copy()
    wk_cat = np.asarray(wk, f32).transpose(1, 0, 2).reshape(c.D, c.D).copy()
    wv_cat = np.asarray(wv, f32).transpose(1, 0, 2).reshape(c.D, c.D).copy()
    wpT = np.asarray(w_proj, f32).T.copy()       # [D_in(cat), D_out]
    # gate computed from the normalized x2n: fold ln2_g into the weights,
    # ln2_b contributes a per-expert constant
    ln2_g_ = np.asarray(ln2_g, f32)
    ln2_b_ = np.asarray(ln2_b, f32)
    gw = np.asarray(gate_w, f32)
    gwT = np.ascontiguousarray((gw * ln2_g_[None, :]).T)
    gconst = (gw @ ln2_b_)[None, :].astype(f32)
    posT = np.ascontiguousarray((rot + pemb).T.astype(f32))
    wq_r = _wtile(wq_cat, P, c.DC, c.H // 2).astype(bf16)
    wk_r = _wtile(wk_cat, P, c.DC, c.H // 2).astype(bf16)
    wv_r = _wtile(wv_cat, P, c.DC, c.H // 2).astype(bf16)

    # diag-block causal mask: key (kl*128+r) visible to query q iff
    # kl*128+r <= q  (same pattern for every 512-query slice)
    kk = np.arange(512)
    maskD = np.where(kk[:, None] <= kk[None, :], 1.0, 0.0).astype(bf16)

    in_maps = []
    for core in range(NCORE):
        b, g = core // (NCORE // c.B), core % (NCORE // c.B)
        t0 = g * c.TL
        onehot = np.zeros((1, c.E), f32)
        onehot[0, core % c.E] = 1.0
        xposT = (x[b].T + posT).astype(bf16)
        gtok = (np.arange(c.TL) + core * c.TL).astype(np.float64)
        # per-core w_proj.T pack over 16 ci chunks: ci chunk cchunk =
        # pair p_=cchunk//8, rank i=cchunk%8 -> cat feature rows
        # 256*(i%4) + 128*p_.  Foreign batch group -> zeros.
        W16 = np.zeros((c.DC, P, 2 * c.DC, P), f32)
        for cchunk in range(2 * c.DC):
            p_, i = cchunk // 8, cchunk % 8
            if i // 4 != core // 4:
                continue
            feat = 256 * (i % 4) + 128 * p_
            for o in range(c.DC):
                W16[o, :, cchunk, :] = wpT[feat:feat + P,
                                           o * P:(o + 1) * P]
        m = {
            "xposT": np.ascontiguousarray(xposT),
            "xqT": x[b, t0:t0 + c.TL].T.copy(),
            "wq": np.ascontiguousarray(wq_r[2 * g:2 * g + 2]),
            "wk": np.ascontiguousarray(wk_r[2 * g:2 * g + 2]),
            "wv": np.ascontiguousarray(wv_r[2 * g:2 * g + 2]),
            "wpT": W16.astype(bf16),
            "bproj": np.asarray(b_proj, f32),
            "ln1g": np.asarray(ln1_g, f32), "ln1b": np.asarray(ln1_b, f32),
            "ln2g": np.asarray(ln2_g, f32), "ln2b": np.asarray(ln2_b, f32),
            "gwT": gwT,
            "w1": _wtile(np.asarray(e_w1, f32)[core % c.E], P, c.DC,
                         c.FT).astype(bf16),
            "b1": np.asarray(e_b1, f32)[core % c.E].copy(),
            "w2": np.ascontiguousarray(
                np.asarray(e_w2, f32)[core % c.E]).astype(bf16),
            "b2": np.asarray(e_b2, f32)[core % c.E].copy(),
            "maskD": maskD,
            "onehot": onehot,
            "gconst": gconst,
            "tok2048": (gtok * 2048.0 + 1.0).astype(f32),
        }
        in_maps.append(m)
    return in_maps


_CACHE = {}
LAST_RESULTS = None


def _ensure_ntff_hook():
    """Inject antenv.axon_hooks (missing from this image) and install the
    ctypes NTFF profile hook against libaxon_pjrt.so so that
    run_bass_kernel_spmd(trace=True) can capture device profiles."""
    import contextlib
    import ctypes
    import types

    try:
        from antenv.axon_hooks import get_axon_ntff_profile_hook  # noqa: F401
        return True
    except ImportError:
        pass
    so_path = "/opt/axon/libaxon_pjrt.so"
    if not os.path.exists(so_path):
        return False
    lib = ctypes.CDLL(so_path)
    if not hasattr(lib, "axon_start_nrt_profile"):
        return False
    lib.axon_start_nrt_profile.argtypes = [ctypes.POINTER(ctypes.c_int64),
                                           ctypes.c_size_t]
    lib.axon_start_nrt_profile.restype = ctypes.c_int64
    lib.axon_stop_nrt_profile.argtypes = [ctypes.c_char_p]
    lib.axon_stop_nrt_profile.restype = ctypes.c_int64

    @contextlib.contextmanager
    def _hook(output_dir, device_ids):
        import jax
        jax.devices()
        if device_ids:
            ids = (ctypes.c_int64 * len(device_ids))(*device_ids)
            rc = lib.axon_start_nrt_profile(ids, len(device_ids))
        else:
            rc = lib.axon_start_nrt_profile(None, 0)
        if rc != 0:
            raise RuntimeError(f"axon_start_nrt_profile rc={rc}")
        try:
            yield
        finally:
            n = lib.axon_stop_nrt_profile(str(output_dir).encode())
            print(f"ntff profile: {n} file(s) -> {output_dir}",
                  file=sys.stderr)

    mod = types.ModuleType("antenv.axon_hooks")
    state = {"h": _hook}
    mod.set_axon_ntff_profile_hook = lambda h: state.__setitem__("h", h)
    mod.get_axon_ntff_profile_hook = lambda: state["h"]
    sys.modules["antenv.axon_hooks"] = mod
    import antenv
    antenv.axon_hooks = mod
    # avoid remote artifact upload in this container
    from concourse import bass_utils as _bu
    _bu.upload_artifacts = lambda tmpdir: tmpdir
    return True


def kernel(**inputs):
    """Full inputs in (as reference.setup_inputs), full output out."""
    cfg = Cfg()
    key = "full"
    if key not in _CACHE:
        _CACHE[key] = build_nc(cfg)
    nc = _CACHE[key]
    in_maps = make_in_maps(cfg, **{k: np.asarray(v) for k, v in inputs.items()})
    trace = bool(os.environ.get("KB_TRACE"))
    if trace:
        trace = _ensure_ntff_hook()
    from concourse.bass_utils import run_bass_kernel_spmd
    global LAST_RESULTS
    res = run_bass_kernel_spmd(nc, in_maps, list(range(NCORE)), trace=trace)
    LAST_RESULTS = res
    outs = [res.results[i]["out"] for i in range(NCORE)]
    c = cfg
    out = np.zeros((c.B, c.T, c.D), np.float32)
    for core in range(NCORE):
        b, j = core // (NCORE // c.B), core % (NCORE // c.B)
        out[b, j * c.TL:(j + 1) * c.TL] = outs[core]
    return out
